# revision 13
# baseline (speedup 1.0000x reference)
"""Trainium2 Bass kernel for LocalStyleAdaptor (segment-pool + VQ codebook).

Reference computation (see problem):
  x, cnt = segment-mean-pool(ref_mels [B,T,H], mel2ph [B,T]) -> [B,512,H]
  VQ: indices = argmin_j ||x - e_j||^2 ; quantized = E[indices]
  loss = 0.25 * sum(mean((x-q)^2,-1) * np) / sum(np), np = (sum|x|,-1) > 0
  z = x + sg(q - x)  (numerically == quantized)
  perplexity = exp(-sum(p log(p+1e-10))), p = histogram(indices)/ (B*512)

Sharding: data-parallel over batch: 16 batch elems -> 8 cores x 2.
Each core returns z [2,512,256] and stats [1,66] = [hist(64) | loss_num | loss_den].
Host concatenates z, sums stats, computes the two scalars.

Device algorithm (per core, per batch elem):
  Phase 1 (pooling): for each 128-frame chunk, build one-hot A [128t, 512s]
    on DVE (iota == id-1, per-partition scalar compare). Use A slices as
    matmul *weights*, moving operand = [mel_chunk | ones] [128t, 257] -> PSUM
    acc_k [128s, 257] accumulates S (sums) + counts (col 256) over 64 chunks.
  Phase 2 (VQ): cc = max(cnt,1). Dn[s,j] = 2*S.e_j - cc*||e_j||^2 via PE
    (argmax_j Dn == argmin_j of reference distances, scale-free, no division).
    First-index argmax via reduce_max + masked-iota reduce_min (ties like jnp).
    mask1 = exact one-hot of argmin; z rows = mask1^T.T @ E via PE.
    Loss terms from Dn max + counts; hist/num/den via ones^T @ [mask1|contrib|np].
"""

import os
import sys

import numpy as np

for _p in ("/opt/trn_rl_repo", "/root/.axon_site/_ro/trn_rl_repo"):
    if os.path.isdir(_p) and _p not in sys.path:
        sys.path.insert(0, _p)

import concourse.bacc as bacc  # noqa: E402
import concourse.bass as bass  # noqa: E402
import concourse.tile as tile  # noqa: E402
from concourse import mybir  # noqa: E402
from concourse.masks import make_identity  # noqa: E402

F32 = mybir.dt.float32
I32 = mybir.dt.int32
OP = mybir.AluOpType

B, T, H, M = 16, 8192, 256, 64
S = 512          # MAX_PH segments
N_CORES = 8
B_LOC = B // N_CORES   # 2 batch elems per core
N_CHUNK = T // 128     # 64 chunks of 128 frames
MEGA = 4               # chunks per DMA load
BIG = 1000.0
LEVEL = int(os.environ.get("KERNEL_LEVEL", "99"))

_CACHE = {}


def build_nc():
    nc = bacc.Bacc("TRN2", name="lsa_vq")

    mel = nc.dram_tensor("mel", [B_LOC, T, H], F32, kind="ExternalInput")
    ids = nc.dram_tensor("ids", [B_LOC, T], I32, kind="ExternalInput")
    emb = nc.dram_tensor("emb", [M, H], F32, kind="ExternalInput")
    z_out = nc.dram_tensor("z", [B_LOC, S, H], F32, kind="ExternalOutput")
    st_out = nc.dram_tensor("stats", [1, M + 2], F32, kind="ExternalOutput")

    with tile.TileContext(nc) as tc:
        with (
            tc.tile_pool(name="const", bufs=1) as cpool,
            tc.tile_pool(name="melp", bufs=3) as melp,
            tc.tile_pool(name="ohp", bufs=3) as ohp,
            tc.tile_pool(name="ph2", bufs=2) as ph2,
            tc.tile_pool(name="zsb", bufs=2) as zsbp,
            tc.tile_pool(name="acc", bufs=1, space="PSUM") as accp,
            tc.tile_pool(name="misc", bufs=2, space="PSUM") as miscp,
            tc.tile_pool(name="zps", bufs=1, space="PSUM") as zpsp,
            tc.tile_pool(name="stps", bufs=1, space="PSUM") as stpsp,
        ):
            # ---------------- constants / setup ----------------
            ident = cpool.tile([128, 128], F32)
            make_identity(nc, ident[:])

            iota_s_i = cpool.tile([128, S], I32)
            nc.gpsimd.iota(iota_s_i[:], pattern=[[1, S]], base=0, channel_multiplier=0)
            iota_s = cpool.tile([128, S], F32)
            nc.vector.tensor_copy(iota_s[:], iota_s_i[:])

            iota_j_i = cpool.tile([128, M], I32)
            nc.gpsimd.iota(iota_j_i[:], pattern=[[1, M]], base=0, channel_multiplier=0)
            iota_j = cpool.tile([128, M], F32)
            nc.vector.tensor_copy(iota_j[:], iota_j_i[:])

            ones_col = cpool.tile([128, 1], F32)
            nc.vector.memset(ones_col[:], 1.0)
            ones_row = cpool.tile([1, 128], F32)
            nc.vector.memset(ones_row[:], 1.0)

            # embedding + derived tables
            e_sb = cpool.tile([M, H], F32)
            nc.sync.dma_start(out=e_sb[:], in_=emb[:])
            e_sq = cpool.tile([M, H], F32)
            e2_col = cpool.tile([M, 1], F32)   # ||e_j||^2  [64,1]
            nc.scalar.activation(
                e_sq[:], e_sb[:], mybir.ActivationFunctionType.Square,
                accum_out=e2_col[:],
            )
            # E2 as a row then broadcast down 128 partitions (via K=1 matmul)
            e2row_ps = miscp.tile([1, M], F32, tag="tp")
            nc.tensor.transpose(out=e2row_ps[:], in_=e2_col[:], identity=ident[:M, :M])
            e2_row = cpool.tile([1, M], F32)
            nc.vector.tensor_copy(e2_row[:], e2row_ps[:])
            e2bc_ps = miscp.tile([128, M], F32, tag="tp")
            nc.tensor.matmul(out=e2bc_ps[:], lhsT=ones_row[:], rhs=e2_row[:],
                             start=True, stop=True)
            e2_bc = cpool.tile([128, M], F32)
            nc.vector.tensor_copy(e2_bc[:], e2bc_ps[:])

            # Ep2 = 2 * E^T halves: [128h, 64j] x2
            ep2 = []
            for h in range(2):
                tp = miscp.tile([128, M], F32, tag="tp")
                nc.tensor.transpose(
                    out=tp[:], in_=e_sb[:, h * 128:(h + 1) * 128],
                    identity=ident[:M, :M],
                )
                dst = cpool.tile([128, M], F32, tag=f"ep2_{h}")
                nc.vector.tensor_scalar(dst[:], tp[:], 2.0, None, OP.mult)
                ep2.append(dst)

            # ids, transposed to [128 frame-in-chunk, 64 chunk] as float(id-1)
            ids_t = []
            for b in range(B_LOC):
                idsr = cpool.tile([64, 128], I32, tag=f"idsr_{b}")
                nc.sync.dma_start(
                    out=idsr[:], in_=ids[b].rearrange("(c u) -> c u", c=64)
                )
                idsrf = cpool.tile([64, 128], F32, tag=f"idsrf_{b}")
                nc.vector.tensor_scalar(idsrf[:], idsr[:], -1.0, None, OP.add)
                tp = miscp.tile([128, 64], F32, tag="tp")
                nc.tensor.transpose(out=tp[:], in_=idsrf[:], identity=ident[:64, :64])
                dst = cpool.tile([128, 64], F32, tag=f"idst_{b}")
                nc.vector.tensor_copy(dst[:], tp[:])
                ids_t.append(dst)

            # ---------------- main per-batch work ----------------
            first_stats = [True]
            stats_ps = stpsp.tile([1, M + 2], F32, name="stats_ps") if LEVEL >= 6 else None

            for b in range(B_LOC):
                if LEVEL <= 1:
                    continue
                acc = [
                    accp.tile([128, S], F32, tag=f"acc{k}", name=f"acc{k}_{b}")
                    for k in range(4)
                ]

                # phase 1: pooling
                for mc in range(T // (128 * MEGA)):  # 16 mega chunks
                    melt = melp.tile([128, MEGA, H + 1], F32, tag="melt")
                    nc.sync.dma_start(
                        out=melt[:, :, 0:H],
                        in_=mel[b, mc * 128 * MEGA:(mc + 1) * 128 * MEGA, :]
                        .rearrange("(c u) h -> u c h", u=128),
                    )
                    nc.vector.memset(melt[:, :, H:H + 1], 1.0)
                    for j in range(MEGA):
                        c = mc * MEGA + j
                        a = ohp.tile([128, S], F32, tag="onehot")
                        nc.vector.tensor_scalar(
                            a[:], iota_s[:], ids_t[b][:, c:c + 1], None, OP.is_equal
                        )
                        for k in range(4):
                            nc.tensor.matmul(
                                out=acc[k][:, 0:H + 1],
                                lhsT=a[:, k * 128:(k + 1) * 128],
                                rhs=melt[:, j, :],
                                start=(c == 0), stop=(c == N_CHUNK - 1),
                            )

                # phase 2: VQ + outputs
                zsb = zsbp.tile([128, 4, H], F32, tag="zsb")
                for k in range(4):
                    s_sb = ph2.tile([128, H], F32, tag="s_sb")
                    nc.scalar.copy(s_sb[:], acc[k][:, 0:H])
                    if LEVEL <= 2:
                        nc.vector.tensor_copy(zsb[:, k, :], s_sb[:])
                        continue
                    s_sq = ph2.tile([128, H], F32, tag="s_sq")
                    ss = ph2.tile([128, 1], F32, tag="ss")
                    nc.scalar.activation(
                        s_sq[:], acc[k][:, 0:H],
                        mybir.ActivationFunctionType.Square, accum_out=ss[:],
                    )
                    cc = ph2.tile([128, 1], F32, tag="cc")
                    nc.vector.tensor_scalar(cc[:], acc[k][:, H:H + 1], 1.0, None, OP.max)
                    comb = ph2.tile([128, M + 2], F32, tag="comb")
                    nc.vector.tensor_scalar(
                        comb[:, M + 1:M + 2], acc[k][:, H:H + 1], 1.0, None, OP.min
                    )
                    rec = ph2.tile([128, 1], F32, tag="rec")
                    nc.vector.reciprocal(rec[:], cc[:])

                    # S^T via PE transposes
                    st_sb = ph2.tile([128, H], F32, tag="st_sb")
                    for h in range(2):
                        tp = miscp.tile([128, 128], F32, tag="tp")
                        nc.tensor.transpose(
                            out=tp[:], in_=s_sb[:, h * 128:(h + 1) * 128],
                            identity=ident[:],
                        )
                        nc.scalar.copy(st_sb[:, h * 128:(h + 1) * 128], tp[:])

                    # Dn = 2*S.e - cc*E2   [128s, 64j]
                    dps = miscp.tile([128, M], F32, tag="tp")
                    nc.tensor.matmul(out=dps[:], lhsT=st_sb[:, 0:128], rhs=ep2[0][:],
                                     start=True, stop=False)
                    nc.tensor.matmul(out=dps[:], lhsT=st_sb[:, 128:256], rhs=ep2[1][:],
                                     start=False, stop=True)
                    cce2 = ph2.tile([128, M], F32, tag="cce2")
                    nc.vector.tensor_scalar(cce2[:], e2_bc[:], cc[:, 0:1], None, OP.mult)
                    dn = ph2.tile([128, M], F32, tag="dn")
                    nc.vector.tensor_tensor(dn[:], dps[:], cce2[:], OP.subtract)

                    # argmax with first-index tie-break
                    mx = ph2.tile([128, 1], F32, tag="mx")
                    nc.vector.tensor_reduce(mx[:], dn[:], mybir.AxisListType.X, OP.max)
                    msk0 = ph2.tile([128, M], F32, tag="msk0")
                    nc.vector.tensor_scalar(msk0[:], dn[:], mx[:, 0:1], None, OP.is_equal)
                    pen = ph2.tile([128, M], F32, tag="pen")
                    nc.vector.tensor_scalar(pen[:], msk0[:], -BIG, BIG, OP.mult, OP.add)
                    cand = ph2.tile([128, M], F32, tag="cand")
                    nc.vector.tensor_tensor(cand[:], pen[:], iota_j[:], OP.add)
                    idxf = ph2.tile([128, 1], F32, tag="idxf")
                    nc.vector.tensor_reduce(idxf[:], cand[:], mybir.AxisListType.X, OP.min)
                    nc.vector.tensor_scalar(
                        comb[:, 0:M], iota_j[:], idxf[:, 0:1], None, OP.is_equal
                    )

                    if LEVEL <= 3:
                        nc.vector.tensor_copy(zsb[:, k, 0:M], comb[:, 0:M])
                        continue
                    # e2sel = sum(mask1 * E2)
                    esel_s = ph2.tile([128, M], F32, tag="esel_s")
                    e2sel = ph2.tile([128, 1], F32, tag="e2sel")
                    nc.vector.tensor_tensor(esel_s[:], comb[:, 0:M], e2_bc[:], OP.mult)
                    nc.vector.tensor_reduce(e2sel[:], esel_s[:], mybir.AxisListType.X, OP.add)

                    # e_lat = (SS*rec^2 - (mx + cc*e2sel)*rec + e2sel)/256
                    u1 = ph2.tile([128, 1], F32, tag="u1")
                    nc.vector.tensor_tensor(u1[:], cc[:], e2sel[:], OP.mult)
                    u2 = ph2.tile([128, 1], F32, tag="u2")
                    nc.vector.tensor_tensor(u2[:], u1[:], mx[:], OP.add)
                    b3 = ph2.tile([128, 1], F32, tag="b3")
                    nc.vector.tensor_tensor(b3[:], u2[:], rec[:], OP.mult)
                    a1 = ph2.tile([128, 1], F32, tag="a1")
                    nc.vector.tensor_tensor(a1[:], ss[:], rec[:], OP.mult)
                    a2 = ph2.tile([128, 1], F32, tag="a2")
                    nc.vector.tensor_tensor(a2[:], a1[:], rec[:], OP.mult)
                    c1 = ph2.tile([128, 1], F32, tag="c1")
                    nc.vector.tensor_tensor(c1[:], a2[:], b3[:], OP.subtract)
                    c2 = ph2.tile([128, 1], F32, tag="c2")
                    nc.vector.tensor_tensor(c2[:], c1[:], e2sel[:], OP.add)
                    c3 = ph2.tile([128, 1], F32, tag="c3")
                    nc.vector.tensor_scalar(c3[:], c2[:], 1.0 / H, None, OP.mult)
                    nc.vector.tensor_tensor(
                        comb[:, M:M + 1], c3[:], comb[:, M + 1:M + 2], OP.mult
                    )

                    if LEVEL <= 4:
                        nc.vector.tensor_copy(zsb[:, k, 0:2], comb[:, M:M + 2])
                        continue
                    # z rows = one_hot(idx) @ E
                    m1t_ps = miscp.tile([M, 128], F32, tag="tp")
                    nc.tensor.transpose(out=m1t_ps[:], in_=comb[:, 0:M], identity=ident[:])
                    m1t_sb = ph2.tile([M, 128], F32, tag="m1t_sb")
                    nc.scalar.copy(m1t_sb[:], m1t_ps[:])
                    zps = zpsp.tile([128, H], F32, tag="zps")
                    nc.tensor.matmul(out=zps[:], lhsT=m1t_sb[:], rhs=e_sb[:],
                                     start=True, stop=True)
                    nc.scalar.copy(zsb[:, k, :], zps[:])

                    if LEVEL <= 5:
                        continue
                    # stats accumulation: [hist(64) | num | den]
                    last = (b == B_LOC - 1) and (k == 3)
                    nc.tensor.matmul(out=stats_ps[:], lhsT=ones_col[:], rhs=comb[:],
                                     start=first_stats[0], stop=last)
                    first_stats[0] = False

                nc.sync.dma_start(
                    out=z_out[b].rearrange("(k u) h -> u k h", k=4), in_=zsb[:]
                )

            if LEVEL >= 6:
                stats_sb = cpool.tile([1, M + 2], F32)
                nc.vector.tensor_copy(stats_sb[:], stats_ps[:])
                nc.sync.dma_start(out=st_out[:], in_=stats_sb[:])

    nc.finalize()
    return nc


def _get_nc():
    if "nc" not in _CACHE:
        _CACHE["nc"] = build_nc()
    return _CACHE["nc"]


def _ensure_axon_profile_hook():
    """Register the NTFF profile hook that this image's antenv lacks."""
    try:
        from antenv.axon_hooks import get_axon_ntff_profile_hook  # noqa: F401
        return
    except ImportError:
        pass
    import types

    import antenv

    mod = types.ModuleType("antenv.axon_hooks")
    _h = {"hook": None}

    def set_axon_ntff_profile_hook(h):
        _h["hook"] = h

    def get_axon_ntff_profile_hook():
        return _h["hook"]

    mod.set_axon_ntff_profile_hook = set_axon_ntff_profile_hook
    mod.get_axon_ntff_profile_hook = get_axon_ntff_profile_hook
    sys.modules["antenv.axon_hooks"] = mod
    antenv.axon_hooks = mod
    try:
        from trn_agent_boot.trn_boot import _ntff_profile_via_ctypes

        set_axon_ntff_profile_hook(
            _ntff_profile_via_ctypes("/opt/axon/libaxon_pjrt.so")
        )
    except Exception as e:  # degrade to no tracing
        print("ntff hook install failed:", e)


def kernel(ref_mels: np.ndarray, mel2ph: np.ndarray, embedding: np.ndarray):
    from concourse.bass_utils import run_bass_kernel_spmd

    nc = _get_nc()

    ref_mels = np.ascontiguousarray(ref_mels, dtype=np.float32)
    mel2ph = np.ascontiguousarray(mel2ph, dtype=np.int32)
    embedding = np.ascontiguousarray(embedding, dtype=np.float32)

    in_maps = []
    for c in range(N_CORES):
        in_maps.append({
            "mel": ref_mels[c * B_LOC:(c + 1) * B_LOC],
            "ids": mel2ph[c * B_LOC:(c + 1) * B_LOC],
            "emb": embedding,
        })

    trace = bool(int(os.environ.get("KERNEL_TRACE", "0")))
    if trace:
        _ensure_axon_profile_hook()
    res = run_bass_kernel_spmd(
        nc, in_maps, core_ids=list(range(N_CORES)), trace=trace,
    )
    _CACHE["last_results"] = res

    z = np.concatenate([res.results[c]["z"] for c in range(N_CORES)], axis=0)
    stats = np.stack([res.results[c]["stats"][0] for c in range(N_CORES)])
    stats = stats.astype(np.float32)

    hist = stats[:, :M].sum(axis=0, dtype=np.float32)
    num = np.float32(stats[:, M].sum(dtype=np.float32))
    den = np.float32(stats[:, M + 1].sum(dtype=np.float32))
    loss = np.float32(np.float32(0.25) * num / den)

    probs = (hist / np.float32(B * S)).astype(np.float32)
    plogp = probs * np.log(probs + np.float32(1e-10), dtype=np.float32)
    perplexity = np.exp(-plogp.sum(dtype=np.float32)).astype(np.float32)

    return z, np.asarray(loss, np.float32), np.asarray(perplexity, np.float32)


if __name__ == "__main__":
    nc = build_nc()
    print("built ok")


# revision 15
# speedup vs baseline: 1.5986x; 1.5986x over previous
"""Trainium2 Bass kernel for LocalStyleAdaptor (segment-pool + VQ codebook).

Reference computation (see problem):
  x, cnt = segment-mean-pool(ref_mels [B,T,H], mel2ph [B,T]) -> [B,512,H]
  VQ: indices = argmin_j ||x - e_j||^2 ; quantized = E[indices]
  loss = 0.25 * sum(mean((x-q)^2,-1) * np) / sum(np), np = (sum|x|,-1) > 0
  z = x + sg(q - x)  (numerically == quantized)
  perplexity = exp(-sum(p log(p+1e-10))), p = histogram(indices)/ (B*512)

Sharding: data-parallel over batch: 16 batch elems -> 8 cores x 2.
Each core returns z [2,512,256] and stats [1,66] = [hist(64) | loss_num | loss_den].
Host concatenates z, sums stats, computes the two scalars.

Device algorithm (per core, per batch elem):
  Phase 1 (pooling): for each 128-frame chunk, build one-hot A [128t, 512s]
    on DVE (iota == id-1, per-partition scalar compare). Use A slices as
    matmul *weights*, moving operand = [mel_chunk | ones] [128t, 257] -> PSUM
    acc_k [128s, 257] accumulates S (sums) + counts (col 256) over 64 chunks.
  Phase 2 (VQ): cc = max(cnt,1). Dn[s,j] = 2*S.e_j - cc*||e_j||^2 via PE
    (argmax_j Dn == argmin_j of reference distances, scale-free, no division).
    First-index argmax via reduce_max + masked-iota reduce_min (ties like jnp).
    mask1 = exact one-hot of argmin; z rows = mask1^T.T @ E via PE.
    Loss terms from Dn max + counts; hist/num/den via ones^T @ [mask1|contrib|np].
"""

import os
import sys

import numpy as np

for _p in ("/opt/trn_rl_repo", "/root/.axon_site/_ro/trn_rl_repo"):
    if os.path.isdir(_p) and _p not in sys.path:
        sys.path.insert(0, _p)

import concourse.bacc as bacc  # noqa: E402
import concourse.bass as bass  # noqa: E402
import concourse.tile as tile  # noqa: E402
from concourse import mybir  # noqa: E402
from concourse.masks import make_identity  # noqa: E402

F32 = mybir.dt.float32
F16 = mybir.dt.float16
I32 = mybir.dt.int32
OP = mybir.AluOpType

B, T, H, M = 16, 8192, 256, 64
S = 512          # MAX_PH segments
N_CORES = 8
B_LOC = B // N_CORES   # 2 batch elems per core
N_CHUNK = T // 128     # 64 chunks of 128 frames
MEGA = 4               # chunks per DMA load
BIG = 1000.0
LEVEL = int(os.environ.get("KERNEL_LEVEL", "99"))

_CACHE = {}


def build_nc():
    nc = bacc.Bacc("TRN2", name="lsa_vq")

    mel_hi = nc.dram_tensor("mel_hi", [B_LOC, T, H], F16, kind="ExternalInput")
    mel_lo = nc.dram_tensor("mel_lo", [B_LOC, T, H], F16, kind="ExternalInput")
    ids = nc.dram_tensor("ids", [B_LOC, T], I32, kind="ExternalInput")
    emb = nc.dram_tensor("emb", [M, H], F32, kind="ExternalInput")
    z_out = nc.dram_tensor("z", [B_LOC, S, H], F32, kind="ExternalOutput")
    st_out = nc.dram_tensor("stats", [1, M + 2], F32, kind="ExternalOutput")

    with tile.TileContext(nc) as tc:
        with (
            tc.tile_pool(name="const", bufs=1) as cpool,
            tc.tile_pool(name="melp", bufs=3) as melp,
            tc.tile_pool(name="ohp", bufs=3) as ohp,
            tc.tile_pool(name="ph2", bufs=2) as ph2,
            tc.tile_pool(name="zsb", bufs=2) as zsbp,
            tc.tile_pool(name="acc", bufs=1, space="PSUM") as accp,
            tc.tile_pool(name="misc", bufs=2, space="PSUM") as miscp,
            tc.tile_pool(name="zps", bufs=1, space="PSUM") as zpsp,
            tc.tile_pool(name="stps", bufs=1, space="PSUM") as stpsp,
        ):
            # ---------------- constants / setup ----------------
            ident = cpool.tile([128, 128], F32)
            make_identity(nc, ident[:])

            iota_s_i = cpool.tile([128, S], I32)
            nc.gpsimd.iota(iota_s_i[:], pattern=[[1, S]], base=0, channel_multiplier=0)
            iota_s = cpool.tile([128, S], F16)
            nc.vector.tensor_copy(iota_s[:], iota_s_i[:])

            iota_j_i = cpool.tile([128, M], I32)
            nc.gpsimd.iota(iota_j_i[:], pattern=[[1, M]], base=0, channel_multiplier=0)
            iota_j = cpool.tile([128, M], F32)
            nc.vector.tensor_copy(iota_j[:], iota_j_i[:])

            ones_col = cpool.tile([128, 1], F32)
            nc.vector.memset(ones_col[:], 1.0)
            ones_row = cpool.tile([1, 128], F32)
            nc.vector.memset(ones_row[:], 1.0)

            # embedding + derived tables
            e_sb = cpool.tile([M, H], F32)
            nc.sync.dma_start(out=e_sb[:], in_=emb[:])
            e_sq = cpool.tile([M, H], F32)
            e2_col = cpool.tile([M, 1], F32)   # ||e_j||^2  [64,1]
            nc.scalar.activation(
                e_sq[:], e_sb[:], mybir.ActivationFunctionType.Square,
                accum_out=e2_col[:],
            )
            # E2 as a row then broadcast down 128 partitions (via K=1 matmul)
            e2row_ps = miscp.tile([1, M], F32, tag="tp")
            nc.tensor.transpose(out=e2row_ps[:], in_=e2_col[:], identity=ident[:M, :M])
            e2_row = cpool.tile([1, M], F32)
            nc.vector.tensor_copy(e2_row[:], e2row_ps[:])
            e2bc_ps = miscp.tile([128, M], F32, tag="tp")
            nc.tensor.matmul(out=e2bc_ps[:], lhsT=ones_row[:], rhs=e2_row[:],
                             start=True, stop=True)
            e2_bc = cpool.tile([128, M], F32)
            nc.vector.tensor_copy(e2_bc[:], e2bc_ps[:])

            # Ep2 = 2 * E^T halves: [128h, 64j] x2
            ep2 = []
            for h in range(2):
                tp = miscp.tile([128, M], F32, tag="tp")
                nc.tensor.transpose(
                    out=tp[:], in_=e_sb[:, h * 128:(h + 1) * 128],
                    identity=ident[:M, :M],
                )
                dst = cpool.tile([128, M], F32, tag=f"ep2_{h}")
                nc.vector.tensor_scalar(dst[:], tp[:], 2.0, None, OP.mult)
                ep2.append(dst)

            # ids, transposed to [128 frame-in-chunk, 64 chunk] as float(id-1)
            ids_t = []
            for b in range(B_LOC):
                idsr = cpool.tile([64, 128], I32, tag=f"idsr_{b}")
                nc.sync.dma_start(
                    out=idsr[:], in_=ids[b].rearrange("(c u) -> c u", c=64)
                )
                idsrf = cpool.tile([64, 128], F32, tag=f"idsrf_{b}")
                nc.vector.tensor_scalar(idsrf[:], idsr[:], -1.0, None, OP.add)
                tp = miscp.tile([128, 64], F32, tag="tp")
                nc.tensor.transpose(out=tp[:], in_=idsrf[:], identity=ident[:64, :64])
                dst = cpool.tile([128, 64], F32, tag=f"idst_{b}")
                nc.vector.tensor_copy(dst[:], tp[:])
                ids_t.append(dst)

            # ---------------- main per-batch work ----------------
            first_stats = [True]
            stats_ps = stpsp.tile([1, M + 2], F32, name="stats_ps") if LEVEL >= 6 else None

            for b in range(B_LOC):
                if LEVEL <= 1:
                    continue
                acc = [
                    accp.tile([128, S], F32, tag=f"acc{k}", name=f"acc{k}_{b}")
                    for k in range(4)
                ]

                # phase 1: pooling
                for mc in range(T // (128 * MEGA)):  # 16 mega chunks
                    rows = slice(mc * 128 * MEGA, (mc + 1) * 128 * MEGA)
                    melth = melp.tile([128, MEGA, H + 1], F16, tag="melth")
                    nc.sync.dma_start(
                        out=melth[:, :, 0:H],
                        in_=mel_hi[b, rows, :].rearrange("(c u) h -> u c h", u=128),
                    )
                    nc.vector.memset(melth[:, :, H:H + 1], 1.0)
                    meltl = melp.tile([128, MEGA, H + 1], F16, tag="meltl")
                    nc.sync.dma_start(
                        out=meltl[:, :, 0:H],
                        in_=mel_lo[b, rows, :].rearrange("(c u) h -> u c h", u=128),
                    )
                    nc.vector.memset(meltl[:, :, H:H + 1], 0.0)
                    for j in range(MEGA):
                        c = mc * MEGA + j
                        a = ohp.tile([128, S], F16, tag="onehot")
                        nc.vector.tensor_scalar(
                            a[:], iota_s[:], ids_t[b][:, c:c + 1], None, OP.is_equal
                        )
                        for k in range(4):
                            nc.tensor.matmul(
                                out=acc[k][:, 0:H + 1],
                                lhsT=a[:, k * 128:(k + 1) * 128],
                                rhs=melth[:, j, :],
                                start=(c == 0), stop=False,
                            )
                            nc.tensor.matmul(
                                out=acc[k][:, 0:H + 1],
                                lhsT=a[:, k * 128:(k + 1) * 128],
                                rhs=meltl[:, j, :],
                                start=False, stop=(c == N_CHUNK - 1),
                            )

                # phase 2: VQ + outputs
                zsb = zsbp.tile([128, 4, H], F32, tag="zsb")
                for k in range(4):
                    s_sb = ph2.tile([128, H], F32, tag="s_sb")
                    nc.scalar.copy(s_sb[:], acc[k][:, 0:H])
                    if LEVEL <= 2:
                        nc.vector.tensor_copy(zsb[:, k, :], s_sb[:])
                        continue
                    s_sq = ph2.tile([128, H], F32, tag="s_sq")
                    ss = ph2.tile([128, 1], F32, tag="ss")
                    nc.scalar.activation(
                        s_sq[:], acc[k][:, 0:H],
                        mybir.ActivationFunctionType.Square, accum_out=ss[:],
                    )
                    cc = ph2.tile([128, 1], F32, tag="cc")
                    nc.vector.tensor_scalar(cc[:], acc[k][:, H:H + 1], 1.0, None, OP.max)
                    comb = ph2.tile([128, M + 2], F32, tag="comb")
                    nc.vector.tensor_scalar(
                        comb[:, M + 1:M + 2], acc[k][:, H:H + 1], 1.0, None, OP.min
                    )
                    rec = ph2.tile([128, 1], F32, tag="rec")
                    nc.vector.reciprocal(rec[:], cc[:])

                    # S^T via PE transposes
                    st_sb = ph2.tile([128, H], F32, tag="st_sb")
                    for h in range(2):
                        tp = miscp.tile([128, 128], F32, tag="tp")
                        nc.tensor.transpose(
                            out=tp[:], in_=s_sb[:, h * 128:(h + 1) * 128],
                            identity=ident[:],
                        )
                        nc.scalar.copy(st_sb[:, h * 128:(h + 1) * 128], tp[:])

                    # Dn = 2*S.e - cc*E2   [128s, 64j]
                    dps = miscp.tile([128, M], F32, tag="tp")
                    nc.tensor.matmul(out=dps[:], lhsT=st_sb[:, 0:128], rhs=ep2[0][:],
                                     start=True, stop=False)
                    nc.tensor.matmul(out=dps[:], lhsT=st_sb[:, 128:256], rhs=ep2[1][:],
                                     start=False, stop=True)
                    cce2 = ph2.tile([128, M], F32, tag="cce2")
                    nc.vector.tensor_scalar(cce2[:], e2_bc[:], cc[:, 0:1], None, OP.mult)
                    dn = ph2.tile([128, M], F32, tag="dn")
                    nc.vector.tensor_tensor(dn[:], dps[:], cce2[:], OP.subtract)

                    # argmax with first-index tie-break
                    mx = ph2.tile([128, 1], F32, tag="mx")
                    nc.vector.tensor_reduce(mx[:], dn[:], mybir.AxisListType.X, OP.max)
                    msk0 = ph2.tile([128, M], F32, tag="msk0")
                    nc.vector.tensor_scalar(msk0[:], dn[:], mx[:, 0:1], None, OP.is_equal)
                    pen = ph2.tile([128, M], F32, tag="pen")
                    nc.vector.tensor_scalar(pen[:], msk0[:], -BIG, BIG, OP.mult, OP.add)
                    cand = ph2.tile([128, M], F32, tag="cand")
                    nc.vector.tensor_tensor(cand[:], pen[:], iota_j[:], OP.add)
                    idxf = ph2.tile([128, 1], F32, tag="idxf")
                    nc.vector.tensor_reduce(idxf[:], cand[:], mybir.AxisListType.X, OP.min)
                    nc.vector.tensor_scalar(
                        comb[:, 0:M], iota_j[:], idxf[:, 0:1], None, OP.is_equal
                    )

                    if LEVEL <= 3:
                        nc.vector.tensor_copy(zsb[:, k, 0:M], comb[:, 0:M])
                        continue
                    # e2sel = sum(mask1 * E2)
                    esel_s = ph2.tile([128, M], F32, tag="esel_s")
                    e2sel = ph2.tile([128, 1], F32, tag="e2sel")
                    nc.vector.tensor_tensor(esel_s[:], comb[:, 0:M], e2_bc[:], OP.mult)
                    nc.vector.tensor_reduce(e2sel[:], esel_s[:], mybir.AxisListType.X, OP.add)

                    # e_lat = (SS*rec^2 - (mx + cc*e2sel)*rec + e2sel)/256
                    u1 = ph2.tile([128, 1], F32, tag="u1")
                    nc.vector.tensor_tensor(u1[:], cc[:], e2sel[:], OP.mult)
                    u2 = ph2.tile([128, 1], F32, tag="u2")
                    nc.vector.tensor_tensor(u2[:], u1[:], mx[:], OP.add)
                    b3 = ph2.tile([128, 1], F32, tag="b3")
                    nc.vector.tensor_tensor(b3[:], u2[:], rec[:], OP.mult)
                    a1 = ph2.tile([128, 1], F32, tag="a1")
                    nc.vector.tensor_tensor(a1[:], ss[:], rec[:], OP.mult)
                    a2 = ph2.tile([128, 1], F32, tag="a2")
                    nc.vector.tensor_tensor(a2[:], a1[:], rec[:], OP.mult)
                    c1 = ph2.tile([128, 1], F32, tag="c1")
                    nc.vector.tensor_tensor(c1[:], a2[:], b3[:], OP.subtract)
                    c2 = ph2.tile([128, 1], F32, tag="c2")
                    nc.vector.tensor_tensor(c2[:], c1[:], e2sel[:], OP.add)
                    c3 = ph2.tile([128, 1], F32, tag="c3")
                    nc.vector.tensor_scalar(c3[:], c2[:], 1.0 / H, None, OP.mult)
                    nc.vector.tensor_tensor(
                        comb[:, M:M + 1], c3[:], comb[:, M + 1:M + 2], OP.mult
                    )

                    if LEVEL <= 4:
                        nc.vector.tensor_copy(zsb[:, k, 0:2], comb[:, M:M + 2])
                        continue
                    # z rows = one_hot(idx) @ E
                    m1t_ps = miscp.tile([M, 128], F32, tag="tp")
                    nc.tensor.transpose(out=m1t_ps[:], in_=comb[:, 0:M], identity=ident[:])
                    m1t_sb = ph2.tile([M, 128], F32, tag="m1t_sb")
                    nc.scalar.copy(m1t_sb[:], m1t_ps[:])
                    zps = zpsp.tile([128, H], F32, tag="zps")
                    nc.tensor.matmul(out=zps[:], lhsT=m1t_sb[:], rhs=e_sb[:],
                                     start=True, stop=True)
                    nc.scalar.copy(zsb[:, k, :], zps[:])

                    if LEVEL <= 5:
                        continue
                    # stats accumulation: [hist(64) | num | den]
                    last = (b == B_LOC - 1) and (k == 3)
                    nc.tensor.matmul(out=stats_ps[:], lhsT=ones_col[:], rhs=comb[:],
                                     start=first_stats[0], stop=last)
                    first_stats[0] = False

                nc.sync.dma_start(
                    out=z_out[b].rearrange("(k u) h -> u k h", k=4), in_=zsb[:]
                )

            if LEVEL >= 6:
                stats_sb = cpool.tile([1, M + 2], F32)
                nc.vector.tensor_copy(stats_sb[:], stats_ps[:])
                nc.sync.dma_start(out=st_out[:], in_=stats_sb[:])

    nc.finalize()
    return nc


def _get_nc():
    if "nc" not in _CACHE:
        _CACHE["nc"] = build_nc()
    return _CACHE["nc"]


def _ensure_axon_profile_hook():
    """Register the NTFF profile hook that this image's antenv lacks."""
    try:
        from antenv.axon_hooks import get_axon_ntff_profile_hook  # noqa: F401
        return
    except ImportError:
        pass
    import types

    import antenv

    mod = types.ModuleType("antenv.axon_hooks")
    _h = {"hook": None}

    def set_axon_ntff_profile_hook(h):
        _h["hook"] = h

    def get_axon_ntff_profile_hook():
        return _h["hook"]

    mod.set_axon_ntff_profile_hook = set_axon_ntff_profile_hook
    mod.get_axon_ntff_profile_hook = get_axon_ntff_profile_hook
    sys.modules["antenv.axon_hooks"] = mod
    antenv.axon_hooks = mod
    try:
        from trn_agent_boot.trn_boot import _ntff_profile_via_ctypes

        set_axon_ntff_profile_hook(
            _ntff_profile_via_ctypes("/opt/axon/libaxon_pjrt.so")
        )
    except Exception as e:  # degrade to no tracing
        print("ntff hook install failed:", e)


def kernel(ref_mels: np.ndarray, mel2ph: np.ndarray, embedding: np.ndarray):
    from concourse.bass_utils import run_bass_kernel_spmd

    nc = _get_nc()

    ref_mels = np.ascontiguousarray(ref_mels, dtype=np.float32)
    mel2ph = np.ascontiguousarray(mel2ph, dtype=np.int32)
    embedding = np.ascontiguousarray(embedding, dtype=np.float32)

    mel_hi = ref_mels.astype(np.float16)
    mel_lo = (ref_mels - mel_hi.astype(np.float32)).astype(np.float16)

    in_maps = []
    for c in range(N_CORES):
        in_maps.append({
            "mel_hi": mel_hi[c * B_LOC:(c + 1) * B_LOC],
            "mel_lo": mel_lo[c * B_LOC:(c + 1) * B_LOC],
            "ids": mel2ph[c * B_LOC:(c + 1) * B_LOC],
            "emb": embedding,
        })

    trace = bool(int(os.environ.get("KERNEL_TRACE", "0")))
    if trace:
        _ensure_axon_profile_hook()
    res = run_bass_kernel_spmd(
        nc, in_maps, core_ids=list(range(N_CORES)), trace=trace,
    )
    _CACHE["last_results"] = res

    z = np.concatenate([res.results[c]["z"] for c in range(N_CORES)], axis=0)
    stats = np.stack([res.results[c]["stats"][0] for c in range(N_CORES)])
    stats = stats.astype(np.float32)

    hist = stats[:, :M].sum(axis=0, dtype=np.float32)
    num = np.float32(stats[:, M].sum(dtype=np.float32))
    den = np.float32(stats[:, M + 1].sum(dtype=np.float32))
    loss = np.float32(np.float32(0.25) * num / den)

    probs = (hist / np.float32(B * S)).astype(np.float32)
    plogp = probs * np.log(probs + np.float32(1e-10), dtype=np.float32)
    perplexity = np.exp(-plogp.sum(dtype=np.float32)).astype(np.float32)

    return z, np.asarray(loss, np.float32), np.asarray(perplexity, np.float32)


if __name__ == "__main__":
    nc = build_nc()
    print("built ok")


# revision 16
# speedup vs baseline: 1.6106x; 1.0075x over previous
"""Trainium2 Bass kernel for LocalStyleAdaptor (segment-pool + VQ codebook).

Reference computation (see problem):
  x, cnt = segment-mean-pool(ref_mels [B,T,H], mel2ph [B,T]) -> [B,512,H]
  VQ: indices = argmin_j ||x - e_j||^2 ; quantized = E[indices]
  loss = 0.25 * sum(mean((x-q)^2,-1) * np) / sum(np), np = (sum|x|,-1) > 0
  z = x + sg(q - x)  (numerically == quantized)
  perplexity = exp(-sum(p log(p+1e-10))), p = histogram(indices)/ (B*512)

Sharding: data-parallel over batch: 16 batch elems -> 8 cores x 2.
Each core returns z [2,512,256] and stats [1,66] = [hist(64) | loss_num | loss_den].
Host concatenates z, sums stats, computes the two scalars.

Device algorithm (per core, per batch elem):
  Phase 1 (pooling): for each 128-frame chunk, build one-hot A [128t, 512s]
    on DVE (iota == id-1, per-partition scalar compare). Use A slices as
    matmul *weights*, moving operand = [mel_chunk | ones] [128t, 257] -> PSUM
    acc_k [128s, 257] accumulates S (sums) + counts (col 256) over 64 chunks.
  Phase 2 (VQ): cc = max(cnt,1). Dn[s,j] = 2*S.e_j - cc*||e_j||^2 via PE
    (argmax_j Dn == argmin_j of reference distances, scale-free, no division).
    First-index argmax via reduce_max + masked-iota reduce_min (ties like jnp).
    mask1 = exact one-hot of argmin; z rows = mask1^T.T @ E via PE.
    Loss terms from Dn max + counts; hist/num/den via ones^T @ [mask1|contrib|np].
"""

import os
import sys

import numpy as np

for _p in ("/opt/trn_rl_repo", "/root/.axon_site/_ro/trn_rl_repo"):
    if os.path.isdir(_p) and _p not in sys.path:
        sys.path.insert(0, _p)

import concourse.bacc as bacc  # noqa: E402
import concourse.bass as bass  # noqa: E402
import concourse.tile as tile  # noqa: E402
from concourse import mybir  # noqa: E402
from concourse.masks import make_identity  # noqa: E402

F32 = mybir.dt.float32
F16 = mybir.dt.float16
I32 = mybir.dt.int32
OP = mybir.AluOpType

B, T, H, M = 16, 8192, 256, 64
S = 512          # MAX_PH segments
N_CORES = 8
B_LOC = B // N_CORES   # 2 batch elems per core
N_CHUNK = T // 128     # 64 chunks of 128 frames
MEGA = 4               # chunks per DMA load
BIG = 1000.0
LEVEL = int(os.environ.get("KERNEL_LEVEL", "99"))

_CACHE = {}


def build_nc():
    nc = bacc.Bacc("TRN2", name="lsa_vq")

    mel_hi = nc.dram_tensor("mel_hi", [B_LOC, T, H], F16, kind="ExternalInput")
    mel_lo = nc.dram_tensor("mel_lo", [B_LOC, T, H], F16, kind="ExternalInput")
    ids = nc.dram_tensor("ids", [B_LOC, T], I32, kind="ExternalInput")
    emb = nc.dram_tensor("emb", [M, H], F32, kind="ExternalInput")
    z_out = nc.dram_tensor("z", [B_LOC, S, H], F32, kind="ExternalOutput")
    st_out = nc.dram_tensor("stats", [1, M + 2], F32, kind="ExternalOutput")

    with tile.TileContext(nc) as tc:
        with (
            tc.tile_pool(name="const", bufs=1) as cpool,
            tc.tile_pool(name="melp", bufs=3) as melp,
            tc.tile_pool(name="ohp", bufs=3) as ohp,
            tc.tile_pool(name="ph2", bufs=2) as ph2,
            tc.tile_pool(name="zsb", bufs=2) as zsbp,
            tc.tile_pool(name="acc", bufs=1, space="PSUM") as accp,
            tc.tile_pool(name="misc", bufs=2, space="PSUM") as miscp,
            tc.tile_pool(name="zps", bufs=1, space="PSUM") as zpsp,
            tc.tile_pool(name="stps", bufs=1, space="PSUM") as stpsp,
        ):
            # ---------------- constants / setup ----------------
            ident = cpool.tile([128, 128], F32)
            make_identity(nc, ident[:])

            iota_s_i = cpool.tile([128, S], I32)
            nc.gpsimd.iota(iota_s_i[:], pattern=[[1, S]], base=0, channel_multiplier=0)
            iota_s = cpool.tile([128, S], F16)
            nc.vector.tensor_copy(iota_s[:], iota_s_i[:])

            iota_j_i = cpool.tile([128, M], I32)
            nc.gpsimd.iota(iota_j_i[:], pattern=[[1, M]], base=0, channel_multiplier=0)
            iota_j = cpool.tile([128, M], F32)
            nc.vector.tensor_copy(iota_j[:], iota_j_i[:])

            ones_col = cpool.tile([128, 1], F32)
            nc.vector.memset(ones_col[:], 1.0)
            ones_row = cpool.tile([1, 128], F32)
            nc.vector.memset(ones_row[:], 1.0)

            # embedding + derived tables
            e_sb = cpool.tile([M, H], F32)
            nc.sync.dma_start(out=e_sb[:], in_=emb[:])
            e_sq = cpool.tile([M, H], F32)
            e2_col = cpool.tile([M, 1], F32)   # ||e_j||^2  [64,1]
            nc.scalar.activation(
                e_sq[:], e_sb[:], mybir.ActivationFunctionType.Square,
                accum_out=e2_col[:],
            )
            # E2 as a row then broadcast down 128 partitions (via K=1 matmul)
            e2row_ps = miscp.tile([1, M], F32, tag="tp")
            nc.tensor.transpose(out=e2row_ps[:], in_=e2_col[:], identity=ident[:M, :M])
            e2_row = cpool.tile([1, M], F32)
            nc.vector.tensor_copy(e2_row[:], e2row_ps[:])
            e2bc_ps = miscp.tile([128, M], F32, tag="tp")
            nc.tensor.matmul(out=e2bc_ps[:], lhsT=ones_row[:], rhs=e2_row[:],
                             start=True, stop=True)
            e2_bc = cpool.tile([128, M], F32)
            nc.vector.tensor_copy(e2_bc[:], e2bc_ps[:])

            # Ep2 = 2 * E^T halves: [128h, 64j] x2
            ep2 = []
            for h in range(2):
                tp = miscp.tile([128, M], F32, tag="tp")
                nc.tensor.transpose(
                    out=tp[:], in_=e_sb[:, h * 128:(h + 1) * 128],
                    identity=ident[:M, :M],
                )
                dst = cpool.tile([128, M], F32, tag=f"ep2_{h}")
                nc.vector.tensor_scalar(dst[:], tp[:], 2.0, None, OP.mult)
                ep2.append(dst)

            # ids, transposed to [128 frame-in-chunk, 64 chunk] as float(id-1)
            ids_t = []
            for b in range(B_LOC):
                idsr = cpool.tile([64, 128], I32, tag=f"idsr_{b}")
                nc.sync.dma_start(
                    out=idsr[:], in_=ids[b].rearrange("(c u) -> c u", c=64)
                )
                idsrf = cpool.tile([64, 128], F32, tag=f"idsrf_{b}")
                nc.vector.tensor_scalar(idsrf[:], idsr[:], -1.0, None, OP.add)
                tp = miscp.tile([128, 64], F32, tag="tp")
                nc.tensor.transpose(out=tp[:], in_=idsrf[:], identity=ident[:64, :64])
                dst = cpool.tile([128, 64], F32, tag=f"idst_{b}")
                nc.vector.tensor_copy(dst[:], tp[:])
                ids_t.append(dst)

            # ---------------- main per-batch work ----------------
            first_stats = [True]
            stats_ps = stpsp.tile([1, M + 2], F32, name="stats_ps") if LEVEL >= 6 else None

            for b in range(B_LOC):
                if LEVEL <= 1:
                    continue
                acc = [
                    accp.tile([128, S], F32, tag=f"acc{k}", name=f"acc{k}_{b}")
                    for k in range(4)
                ]

                # phase 1: pooling
                for mc in range(T // (128 * MEGA)):  # 16 mega chunks
                    rows = slice(mc * 128 * MEGA, (mc + 1) * 128 * MEGA)
                    melth = melp.tile([128, MEGA, H + 1], F16, tag="melth")
                    nc.sync.dma_start(
                        out=melth[:, :, 0:H],
                        in_=mel_hi[b, rows, :].rearrange("(c u) h -> u c h", u=128),
                    )
                    nc.vector.memset(melth[:, :, H:H + 1], 1.0)
                    meltl = melp.tile([128, MEGA, H + 1], F16, tag="meltl")
                    nc.sync.dma_start(
                        out=meltl[:, :, 0:H],
                        in_=mel_lo[b, rows, :].rearrange("(c u) h -> u c h", u=128),
                    )
                    nc.vector.memset(meltl[:, :, H:H + 1], 0.0)
                    for j in range(MEGA):
                        c = mc * MEGA + j
                        a = ohp.tile([128, S], F16, tag="onehot")
                        nc.vector.tensor_scalar(
                            a[:], iota_s[:], ids_t[b][:, c:c + 1], None, OP.is_equal
                        )
                        for k in range(4):
                            nc.tensor.matmul(
                                out=acc[k][:, 0:H + 1],
                                lhsT=a[:, k * 128:(k + 1) * 128],
                                rhs=melth[:, j, :],
                                start=(c == 0), stop=False,
                            )
                            nc.tensor.matmul(
                                out=acc[k][:, 0:H + 1],
                                lhsT=a[:, k * 128:(k + 1) * 128],
                                rhs=meltl[:, j, :],
                                start=False, stop=(c == N_CHUNK - 1),
                            )

                # phase 2a: drain PSUM accumulators ASAP (frees banks for
                # the next batch's pooling matmuls)
                zsb = zsbp.tile([128, 4, H], F32, tag="zsb")
                s_sbs, sss, ccs, combs, recs = [], [], [], [], []
                for k in range(4):
                    s_sb = ph2.tile([128, H], F32, tag="s_sb", bufs=8)
                    nc.scalar.copy(s_sb[:], acc[k][:, 0:H])
                    s_sbs.append(s_sb)
                    if LEVEL <= 2:
                        nc.vector.tensor_copy(zsb[:, k, :], s_sb[:])
                        continue
                    s_sq = ph2.tile([128, H], F32, tag="s_sq")
                    ss = ph2.tile([128, 1], F32, tag="ss", bufs=8)
                    nc.scalar.activation(
                        s_sq[:], acc[k][:, 0:H],
                        mybir.ActivationFunctionType.Square, accum_out=ss[:],
                    )
                    sss.append(ss)
                    cc = ph2.tile([128, 1], F32, tag="cc", bufs=8)
                    nc.vector.tensor_scalar(cc[:], acc[k][:, H:H + 1], 1.0, None, OP.max)
                    ccs.append(cc)
                    comb = ph2.tile([128, M + 2], F32, tag="comb", bufs=8)
                    nc.vector.tensor_scalar(
                        comb[:, M + 1:M + 2], acc[k][:, H:H + 1], 1.0, None, OP.min
                    )
                    combs.append(comb)
                    rec = ph2.tile([128, 1], F32, tag="rec", bufs=8)
                    nc.vector.reciprocal(rec[:], cc[:])
                    recs.append(rec)

                if LEVEL <= 2:
                    nc.sync.dma_start(
                        out=z_out[b].rearrange("(k u) h -> u k h", k=4), in_=zsb[:]
                    )
                    continue

                # phase 2b: VQ chains per s-tile
                for k in range(4):
                    s_sb, ss, cc, comb, rec = (
                        s_sbs[k], sss[k], ccs[k], combs[k], recs[k]
                    )
                    # S^T via PE transposes
                    st_sb = ph2.tile([128, H], F32, tag="st_sb")
                    for h in range(2):
                        tp = miscp.tile([128, 128], F32, tag="tp")
                        nc.tensor.transpose(
                            out=tp[:], in_=s_sb[:, h * 128:(h + 1) * 128],
                            identity=ident[:],
                        )
                        nc.scalar.copy(st_sb[:, h * 128:(h + 1) * 128], tp[:])

                    # Dn = 2*S.e - cc*E2   [128s, 64j]
                    dps = miscp.tile([128, M], F32, tag="tp")
                    nc.tensor.matmul(out=dps[:], lhsT=st_sb[:, 0:128], rhs=ep2[0][:],
                                     start=True, stop=False)
                    nc.tensor.matmul(out=dps[:], lhsT=st_sb[:, 128:256], rhs=ep2[1][:],
                                     start=False, stop=True)
                    cce2 = ph2.tile([128, M], F32, tag="cce2")
                    nc.vector.tensor_scalar(cce2[:], e2_bc[:], cc[:, 0:1], None, OP.mult)
                    dn = ph2.tile([128, M], F32, tag="dn")
                    nc.vector.tensor_tensor(dn[:], dps[:], cce2[:], OP.subtract)

                    # argmax with first-index tie-break
                    mx = ph2.tile([128, 1], F32, tag="mx")
                    nc.vector.tensor_reduce(mx[:], dn[:], mybir.AxisListType.X, OP.max)
                    msk0 = ph2.tile([128, M], F32, tag="msk0")
                    nc.vector.tensor_scalar(msk0[:], dn[:], mx[:, 0:1], None, OP.is_equal)
                    pen = ph2.tile([128, M], F32, tag="pen")
                    nc.vector.tensor_scalar(pen[:], msk0[:], -BIG, BIG, OP.mult, OP.add)
                    cand = ph2.tile([128, M], F32, tag="cand")
                    nc.vector.tensor_tensor(cand[:], pen[:], iota_j[:], OP.add)
                    idxf = ph2.tile([128, 1], F32, tag="idxf")
                    nc.vector.tensor_reduce(idxf[:], cand[:], mybir.AxisListType.X, OP.min)
                    nc.vector.tensor_scalar(
                        comb[:, 0:M], iota_j[:], idxf[:, 0:1], None, OP.is_equal
                    )

                    if LEVEL <= 3:
                        nc.vector.tensor_copy(zsb[:, k, 0:M], comb[:, 0:M])
                        continue
                    # e2sel = sum(mask1 * E2)
                    esel_s = ph2.tile([128, M], F32, tag="esel_s")
                    e2sel = ph2.tile([128, 1], F32, tag="e2sel")
                    nc.vector.tensor_tensor(esel_s[:], comb[:, 0:M], e2_bc[:], OP.mult)
                    nc.vector.tensor_reduce(e2sel[:], esel_s[:], mybir.AxisListType.X, OP.add)

                    # e_lat = (SS*rec^2 - (mx + cc*e2sel)*rec + e2sel)/256
                    u1 = ph2.tile([128, 1], F32, tag="u1")
                    nc.vector.tensor_tensor(u1[:], cc[:], e2sel[:], OP.mult)
                    u2 = ph2.tile([128, 1], F32, tag="u2")
                    nc.vector.tensor_tensor(u2[:], u1[:], mx[:], OP.add)
                    b3 = ph2.tile([128, 1], F32, tag="b3")
                    nc.vector.tensor_tensor(b3[:], u2[:], rec[:], OP.mult)
                    a1 = ph2.tile([128, 1], F32, tag="a1")
                    nc.vector.tensor_tensor(a1[:], ss[:], rec[:], OP.mult)
                    a2 = ph2.tile([128, 1], F32, tag="a2")
                    nc.vector.tensor_tensor(a2[:], a1[:], rec[:], OP.mult)
                    c1 = ph2.tile([128, 1], F32, tag="c1")
                    nc.vector.tensor_tensor(c1[:], a2[:], b3[:], OP.subtract)
                    c2 = ph2.tile([128, 1], F32, tag="c2")
                    nc.vector.tensor_tensor(c2[:], c1[:], e2sel[:], OP.add)
                    c3 = ph2.tile([128, 1], F32, tag="c3")
                    nc.vector.tensor_scalar(c3[:], c2[:], 1.0 / H, None, OP.mult)
                    nc.vector.tensor_tensor(
                        comb[:, M:M + 1], c3[:], comb[:, M + 1:M + 2], OP.mult
                    )

                    if LEVEL <= 4:
                        nc.vector.tensor_copy(zsb[:, k, 0:2], comb[:, M:M + 2])
                        continue
                    # z rows = one_hot(idx) @ E
                    m1t_ps = miscp.tile([M, 128], F32, tag="tp")
                    nc.tensor.transpose(out=m1t_ps[:], in_=comb[:, 0:M], identity=ident[:])
                    m1t_sb = ph2.tile([M, 128], F32, tag="m1t_sb")
                    nc.scalar.copy(m1t_sb[:], m1t_ps[:])
                    zps = zpsp.tile([128, H], F32, tag="zps")
                    nc.tensor.matmul(out=zps[:], lhsT=m1t_sb[:], rhs=e_sb[:],
                                     start=True, stop=True)
                    nc.scalar.copy(zsb[:, k, :], zps[:])

                    if LEVEL <= 5:
                        continue
                    # stats accumulation: [hist(64) | num | den]
                    last = (b == B_LOC - 1) and (k == 3)
                    nc.tensor.matmul(out=stats_ps[:], lhsT=ones_col[:], rhs=comb[:],
                                     start=first_stats[0], stop=last)
                    first_stats[0] = False

                nc.sync.dma_start(
                    out=z_out[b].rearrange("(k u) h -> u k h", k=4), in_=zsb[:]
                )

            if LEVEL >= 6:
                stats_sb = cpool.tile([1, M + 2], F32)
                nc.vector.tensor_copy(stats_sb[:], stats_ps[:])
                nc.sync.dma_start(out=st_out[:], in_=stats_sb[:])

    nc.finalize()
    return nc


def _get_nc():
    if "nc" not in _CACHE:
        _CACHE["nc"] = build_nc()
    return _CACHE["nc"]


def _ensure_axon_profile_hook():
    """Register the NTFF profile hook that this image's antenv lacks."""
    try:
        from antenv.axon_hooks import get_axon_ntff_profile_hook  # noqa: F401
        return
    except ImportError:
        pass
    import types

    import antenv

    mod = types.ModuleType("antenv.axon_hooks")
    _h = {"hook": None}

    def set_axon_ntff_profile_hook(h):
        _h["hook"] = h

    def get_axon_ntff_profile_hook():
        return _h["hook"]

    mod.set_axon_ntff_profile_hook = set_axon_ntff_profile_hook
    mod.get_axon_ntff_profile_hook = get_axon_ntff_profile_hook
    sys.modules["antenv.axon_hooks"] = mod
    antenv.axon_hooks = mod
    try:
        from trn_agent_boot.trn_boot import _ntff_profile_via_ctypes

        set_axon_ntff_profile_hook(
            _ntff_profile_via_ctypes("/opt/axon/libaxon_pjrt.so")
        )
    except Exception as e:  # degrade to no tracing
        print("ntff hook install failed:", e)


def kernel(ref_mels: np.ndarray, mel2ph: np.ndarray, embedding: np.ndarray):
    from concourse.bass_utils import run_bass_kernel_spmd

    nc = _get_nc()

    ref_mels = np.ascontiguousarray(ref_mels, dtype=np.float32)
    mel2ph = np.ascontiguousarray(mel2ph, dtype=np.int32)
    embedding = np.ascontiguousarray(embedding, dtype=np.float32)

    mel_hi = ref_mels.astype(np.float16)
    mel_lo = (ref_mels - mel_hi.astype(np.float32)).astype(np.float16)

    in_maps = []
    for c in range(N_CORES):
        in_maps.append({
            "mel_hi": mel_hi[c * B_LOC:(c + 1) * B_LOC],
            "mel_lo": mel_lo[c * B_LOC:(c + 1) * B_LOC],
            "ids": mel2ph[c * B_LOC:(c + 1) * B_LOC],
            "emb": embedding,
        })

    trace = bool(int(os.environ.get("KERNEL_TRACE", "0")))
    if trace:
        _ensure_axon_profile_hook()
    res = run_bass_kernel_spmd(
        nc, in_maps, core_ids=list(range(N_CORES)), trace=trace,
    )
    _CACHE["last_results"] = res

    z = np.concatenate([res.results[c]["z"] for c in range(N_CORES)], axis=0)
    stats = np.stack([res.results[c]["stats"][0] for c in range(N_CORES)])
    stats = stats.astype(np.float32)

    hist = stats[:, :M].sum(axis=0, dtype=np.float32)
    num = np.float32(stats[:, M].sum(dtype=np.float32))
    den = np.float32(stats[:, M + 1].sum(dtype=np.float32))
    loss = np.float32(np.float32(0.25) * num / den)

    probs = (hist / np.float32(B * S)).astype(np.float32)
    plogp = probs * np.log(probs + np.float32(1e-10), dtype=np.float32)
    perplexity = np.exp(-plogp.sum(dtype=np.float32)).astype(np.float32)

    return z, np.asarray(loss, np.float32), np.asarray(perplexity, np.float32)


if __name__ == "__main__":
    nc = build_nc()
    print("built ok")


# revision 17
# speedup vs baseline: 1.6579x; 1.0293x over previous
"""Trainium2 Bass kernel for LocalStyleAdaptor (segment-pool + VQ codebook).

Reference computation:
  x, cnt = segment-mean-pool(ref_mels [B,T,H], mel2ph [B,T]) -> [B,512,H]
  VQ: indices = argmin_j ||x - e_j||^2 ; quantized = E[indices]
  loss = 0.25 * sum(mean((x-q)^2,-1) * np) / sum(np), np = (sum|x|,-1) > 0
  z = x + sg(q - x)  (numerically == quantized)
  perplexity = exp(-sum(p log(p+1e-10))), p = histogram(indices)/(B*512)

Sharding: data-parallel over batch: 16 batch elems -> 8 cores x 2.
Each core returns z [2,512,256] and stats [1,66] = [hist(64) | loss_num | loss_den].
Host concatenates z, sums stats, computes the two scalars.

Device algorithm (per core, per batch elem):
  Phase 1 (pooling): mel is pre-split on host into fp16 hi + fp16 lo
    (hi+lo reconstructs fp32 to ~22 mantissa bits; fp16 streams through the
    PE at 1 col/cycle vs fp32's effective 1/4). For each 128-frame chunk,
    build one-hot A [128t, 512s] in fp16 on DVE (iota == id-1, per-partition
    scalar compare). Use A slices as matmul *weights*; two moving passes
    [mel_hi | ones] and [mel_lo | zeros] [128t, 257] accumulate into PSUM
    acc_k [128s, 257]: S (sums) + counts (col 256) over 64 chunks.
  Phase 2a (drain): copy S/counts out of PSUM immediately so the next
    batch's pooling can reuse the accumulator banks.
  Phase 2b (VQ): Dn[s,j] = 2*S.e_j - max(cnt,1)*||e_j||^2 via PE
    (argmax_j Dn == argmin_j of reference distances, scale-free).
    First-index argmax via reduce_max + masked-iota reduce_min (ties as jnp).
    mask1 = exact one-hot of argmin; z rows = mask1^T.T @ E via PE.
  Phase 2c: loss columns batched [128,4]; stats accumulated in SBUF,
    one final ones^T @ statacc matmul -> [1,66].
"""

import os
import sys

import numpy as np

for _p in ("/opt/trn_rl_repo", "/root/.axon_site/_ro/trn_rl_repo"):
    if os.path.isdir(_p) and _p not in sys.path:
        sys.path.insert(0, _p)

import concourse.bacc as bacc  # noqa: E402
import concourse.tile as tile  # noqa: E402
from concourse import mybir  # noqa: E402
from concourse.masks import make_identity  # noqa: E402

F32 = mybir.dt.float32
F16 = mybir.dt.float16
I32 = mybir.dt.int32
OP = mybir.AluOpType
AX = mybir.AxisListType
AF = mybir.ActivationFunctionType

B, T, H, M = 16, 8192, 256, 64
S = 512          # MAX_PH segments
N_CORES = 8
B_LOC = B // N_CORES   # 2 batch elems per core
N_CHUNK = T // 128     # 64 chunks of 128 frames
MEGA = 4               # chunks per DMA load
BIG = 1000.0
LEVEL = int(os.environ.get("KERNEL_LEVEL", "99"))

_CACHE = {}


def build_nc():
    nc = bacc.Bacc("TRN2", name="lsa_vq")

    mel_hi = nc.dram_tensor("mel_hi", [B_LOC, T, H], F16, kind="ExternalInput")
    mel_lo = nc.dram_tensor("mel_lo", [B_LOC, T, H], F16, kind="ExternalInput")
    ids = nc.dram_tensor("ids", [B_LOC, T], I32, kind="ExternalInput")
    emb = nc.dram_tensor("emb", [M, H], F32, kind="ExternalInput")
    z_out = nc.dram_tensor("z", [B_LOC, S, H], F32, kind="ExternalOutput")
    st_out = nc.dram_tensor("stats", [1, M + 2], F32, kind="ExternalOutput")

    with tile.TileContext(nc) as tc:
        with (
            tc.tile_pool(name="const", bufs=1) as cpool,
            tc.tile_pool(name="melp", bufs=3) as melp,
            tc.tile_pool(name="ohp", bufs=3) as ohp,
            tc.tile_pool(name="ph2", bufs=2) as ph2,
            tc.tile_pool(name="zsb", bufs=2) as zsbp,
            tc.tile_pool(name="acc", bufs=1, space="PSUM") as accp,
            tc.tile_pool(name="misc", bufs=3, space="PSUM") as miscp,
            tc.tile_pool(name="zps", bufs=1, space="PSUM") as zpsp,
        ):
            # ---------------- ids first (phase-1 critical path) ----------
            ident = cpool.tile([128, 128], F32)
            make_identity(nc, ident[:])

            idsrf_l = []
            for b in range(B_LOC):
                idsr = cpool.tile([64, 128], I32, tag=f"idsr_{b}", name=f"idsr{b}")
                nc.sync.dma_start(
                    out=idsr[:], in_=ids[b].rearrange("(c u) -> c u", c=64)
                )
                idsrf = cpool.tile([64, 128], F32, tag=f"idsrf_{b}", name=f"idsrf{b}")
                nc.vector.tensor_scalar(idsrf[:], idsr[:], -1.0, None, OP.add)
                idsrf_l.append(idsrf)

            ids_t = []
            for b in range(B_LOC):
                tp = miscp.tile([128, 64], F32, tag="tp", name=f"idtp{b}")
                nc.tensor.transpose(
                    out=tp[:], in_=idsrf_l[b][:], identity=ident[:64, :64]
                )
                dst = cpool.tile([128, 64], F32, tag=f"idst_{b}", name=f"idst{b}")
                nc.vector.tensor_copy(dst[:], tp[:])
                ids_t.append(dst)

            iota_s_i = cpool.tile([128, S], I32)
            nc.gpsimd.iota(iota_s_i[:], pattern=[[1, S]], base=0, channel_multiplier=0)
            iota_s = cpool.tile([128, S], F16)
            nc.vector.tensor_copy(iota_s[:], iota_s_i[:])

            iota_j_i = cpool.tile([128, M], I32)
            nc.gpsimd.iota(iota_j_i[:], pattern=[[1, M]], base=0, channel_multiplier=0)
            iota_j = cpool.tile([128, M], F32)
            nc.vector.tensor_copy(iota_j[:], iota_j_i[:])

            ones_col = cpool.tile([128, 1], F32)
            nc.vector.memset(ones_col[:], 1.0)
            ones_row = cpool.tile([1, 128], F32)
            nc.vector.memset(ones_row[:], 1.0)
            statacc = cpool.tile([128, M + 2], F32)
            nc.vector.memset(statacc[:], 0.0)

            # ---------------- embedding tables ----------------
            e_sb = cpool.tile([M, H], F32)
            nc.sync.dma_start(out=e_sb[:], in_=emb[:])
            e_sq = cpool.tile([M, H], F32)
            e2_col = cpool.tile([M, 1], F32)   # ||e_j||^2  [64,1]
            nc.scalar.activation(e_sq[:], e_sb[:], AF.Square, accum_out=e2_col[:])
            e2row_ps = miscp.tile([1, M], F32, tag="tp")
            nc.tensor.transpose(out=e2row_ps[:], in_=e2_col[:], identity=ident[:M, :M])
            e2_row = cpool.tile([1, M], F32)
            nc.vector.tensor_copy(e2_row[:], e2row_ps[:])
            e2bc_ps = miscp.tile([128, M], F32, tag="tp")
            nc.tensor.matmul(out=e2bc_ps[:], lhsT=ones_row[:], rhs=e2_row[:],
                             start=True, stop=True)
            e2_bc = cpool.tile([128, M], F32)
            nc.vector.tensor_copy(e2_bc[:], e2bc_ps[:])

            # Ep2 = 2 * E^T halves: [128h, 64j] x2
            ep2 = []
            for h in range(2):
                tp = miscp.tile([128, M], F32, tag="tp", name=f"etp{h}")
                nc.tensor.transpose(
                    out=tp[:], in_=e_sb[:, h * 128:(h + 1) * 128],
                    identity=ident[:M, :M],
                )
                dst = cpool.tile([128, M], F32, tag=f"ep2_{h}", name=f"ep2{h}")
                nc.vector.tensor_scalar(dst[:], tp[:], 2.0, None, OP.mult)
                ep2.append(dst)

            # ---------------- main per-batch work ----------------
            for b in range(B_LOC):
                acc = [
                    accp.tile([128, S], F32, tag=f"acc{k}", name=f"acc{k}_{b}")
                    for k in range(4)
                ]

                # phase 1: pooling
                for mc in range(T // (128 * MEGA)):  # 16 mega chunks
                    rows = slice(mc * 128 * MEGA, (mc + 1) * 128 * MEGA)
                    melth = melp.tile([128, MEGA, H + 1], F16, tag="melth")
                    nc.sync.dma_start(
                        out=melth[:, :, 0:H],
                        in_=mel_hi[b, rows, :].rearrange("(c u) h -> u c h", u=128),
                    )
                    nc.vector.memset(melth[:, :, H:H + 1], 1.0)
                    meltl = melp.tile([128, MEGA, H + 1], F16, tag="meltl")
                    nc.sync.dma_start(
                        out=meltl[:, :, 0:H],
                        in_=mel_lo[b, rows, :].rearrange("(c u) h -> u c h", u=128),
                    )
                    nc.vector.memset(meltl[:, :, H:H + 1], 0.0)
                    for j in range(MEGA):
                        c = mc * MEGA + j
                        a = ohp.tile([128, S], F16, tag="onehot")
                        nc.vector.tensor_scalar(
                            a[:], iota_s[:], ids_t[b][:, c:c + 1], None, OP.is_equal
                        )
                        for k in range(4):
                            nc.tensor.matmul(
                                out=acc[k][:, 0:H + 1],
                                lhsT=a[:, k * 128:(k + 1) * 128],
                                rhs=melth[:, j, :],
                                start=(c == 0), stop=False,
                            )
                            nc.tensor.matmul(
                                out=acc[k][:, 0:H + 1],
                                lhsT=a[:, k * 128:(k + 1) * 128],
                                rhs=meltl[:, j, :],
                                start=False, stop=(c == N_CHUNK - 1),
                            )

                # phase 2a: drain PSUM accumulators ASAP (frees banks for the
                # next batch's pooling matmuls)
                zsb = zsbp.tile([128, 4, H], F32, tag="zsb")
                s_sbs, combs = [], []
                ccB = ph2.tile([128, 4], F32, tag="ccB")
                ssB = ph2.tile([128, 4], F32, tag="ssB")
                recB = ph2.tile([128, 4], F32, tag="recB")
                mxB = ph2.tile([128, 4], F32, tag="mxB")
                e2selB = ph2.tile([128, 4], F32, tag="e2selB")
                for k in range(4):
                    s_sb = ph2.tile([128, H], F32, tag="s_sb", bufs=8)
                    nc.scalar.copy(s_sb[:], acc[k][:, 0:H])
                    s_sbs.append(s_sb)
                    if LEVEL <= 2:
                        nc.vector.tensor_copy(zsb[:, k, :], s_sb[:])
                        continue
                    s_sq = ph2.tile([128, H], F32, tag="s_sq")
                    nc.scalar.activation(
                        s_sq[:], acc[k][:, 0:H], AF.Square,
                        accum_out=ssB[:, k:k + 1],
                    )
                    nc.vector.tensor_scalar(
                        ccB[:, k:k + 1], acc[k][:, H:H + 1], 1.0, None, OP.max
                    )
                    comb = ph2.tile([128, M + 2], F32, tag="comb", bufs=8)
                    nc.vector.tensor_scalar(
                        comb[:, M + 1:M + 2], acc[k][:, H:H + 1], 1.0, None, OP.min
                    )
                    combs.append(comb)

                if LEVEL <= 2:
                    nc.sync.dma_start(
                        out=z_out[b].rearrange("(k u) h -> u k h", k=4), in_=zsb[:]
                    )
                    continue

                nc.vector.reciprocal(recB[:], ccB[:])

                # phase 2b: VQ chain per s-tile
                for k in range(4):
                    s_sb, comb = s_sbs[k], combs[k]
                    st_sb = ph2.tile([128, H], F32, tag="st_sb")
                    for h in range(2):
                        tp = miscp.tile([128, 128], F32, tag="tp", name=f"stp{h}")
                        nc.tensor.transpose(
                            out=tp[:], in_=s_sb[:, h * 128:(h + 1) * 128],
                            identity=ident[:],
                        )
                        nc.scalar.copy(st_sb[:, h * 128:(h + 1) * 128], tp[:])

                    dps = miscp.tile([128, M], F32, tag="tp")
                    nc.tensor.matmul(out=dps[:], lhsT=st_sb[:, 0:128], rhs=ep2[0][:],
                                     start=True, stop=False)
                    nc.tensor.matmul(out=dps[:], lhsT=st_sb[:, 128:256], rhs=ep2[1][:],
                                     start=False, stop=True)
                    cce2 = ph2.tile([128, M], F32, tag="cce2")
                    nc.vector.tensor_scalar(
                        cce2[:], e2_bc[:], ccB[:, k:k + 1], None, OP.mult
                    )
                    dn = ph2.tile([128, M], F32, tag="dn")
                    nc.vector.tensor_tensor(dn[:], dps[:], cce2[:], OP.subtract)

                    # argmax with first-index tie-break
                    nc.vector.tensor_reduce(mxB[:, k:k + 1], dn[:], AX.X, OP.max)
                    msk0 = ph2.tile([128, M], F32, tag="msk0")
                    nc.vector.tensor_scalar(
                        msk0[:], dn[:], mxB[:, k:k + 1], None, OP.is_equal
                    )
                    pen = ph2.tile([128, M], F32, tag="pen")
                    nc.vector.tensor_scalar(pen[:], msk0[:], -BIG, BIG, OP.mult, OP.add)
                    cand = ph2.tile([128, M], F32, tag="cand")
                    nc.vector.tensor_tensor(cand[:], pen[:], iota_j[:], OP.add)
                    idxf = ph2.tile([128, 1], F32, tag="idxf")
                    nc.vector.tensor_reduce(idxf[:], cand[:], AX.X, OP.min)
                    nc.vector.tensor_scalar(
                        comb[:, 0:M], iota_j[:], idxf[:, 0:1], None, OP.is_equal
                    )

                    # e2sel = sum(mask1 * E2)
                    esel_s = ph2.tile([128, M], F32, tag="esel_s")
                    nc.vector.tensor_tensor(esel_s[:], comb[:, 0:M], e2_bc[:], OP.mult)
                    nc.vector.tensor_reduce(e2selB[:, k:k + 1], esel_s[:], AX.X, OP.add)

                    # z rows = one_hot(idx) @ E
                    m1t_ps = miscp.tile([M, 128], F32, tag="tp")
                    nc.tensor.transpose(out=m1t_ps[:], in_=comb[:, 0:M],
                                        identity=ident[:])
                    m1t_sb = ph2.tile([M, 128], F32, tag="m1t_sb")
                    nc.scalar.copy(m1t_sb[:], m1t_ps[:])
                    zps = zpsp.tile([128, H], F32, tag="zps")
                    nc.tensor.matmul(out=zps[:], lhsT=m1t_sb[:], rhs=e_sb[:],
                                     start=True, stop=True)
                    nc.scalar.copy(zsb[:, k, :], zps[:])

                nc.sync.dma_start(
                    out=z_out[b].rearrange("(k u) h -> u k h", k=4), in_=zsb[:]
                )

                # phase 2c: batched loss columns
                # e_lat = (SS*rec^2 - (mx + cc*e2sel)*rec + e2sel)/H
                u1 = ph2.tile([128, 4], F32, tag="u1")
                nc.vector.tensor_tensor(u1[:], ccB[:], e2selB[:], OP.mult)
                u2 = ph2.tile([128, 4], F32, tag="u2")
                nc.vector.tensor_tensor(u2[:], u1[:], mxB[:], OP.add)
                b3 = ph2.tile([128, 4], F32, tag="b3")
                nc.vector.tensor_tensor(b3[:], u2[:], recB[:], OP.mult)
                a1 = ph2.tile([128, 4], F32, tag="a1")
                nc.vector.tensor_tensor(a1[:], ssB[:], recB[:], OP.mult)
                a2 = ph2.tile([128, 4], F32, tag="a2")
                nc.vector.tensor_tensor(a2[:], a1[:], recB[:], OP.mult)
                c1 = ph2.tile([128, 4], F32, tag="c1")
                nc.vector.tensor_tensor(c1[:], a2[:], b3[:], OP.subtract)
                c2 = ph2.tile([128, 4], F32, tag="c2")
                nc.vector.tensor_tensor(c2[:], c1[:], e2selB[:], OP.add)
                c3 = ph2.tile([128, 4], F32, tag="c3")
                nc.vector.tensor_scalar(c3[:], c2[:], 1.0 / H, None, OP.mult)
                for k in range(4):
                    nc.vector.tensor_tensor(
                        combs[k][:, M:M + 1], c3[:, k:k + 1],
                        combs[k][:, M + 1:M + 2], OP.mult,
                    )
                    nc.vector.tensor_tensor(
                        statacc[:], statacc[:], combs[k][:], OP.add
                    )

            if LEVEL >= 3:
                stats_ps = miscp.tile([1, M + 2], F32, tag="tp")
                nc.tensor.matmul(out=stats_ps[:], lhsT=ones_col[:], rhs=statacc[:],
                                 start=True, stop=True)
                stats_sb = cpool.tile([1, M + 2], F32)
                nc.vector.tensor_copy(stats_sb[:], stats_ps[:])
                nc.sync.dma_start(out=st_out[:], in_=stats_sb[:])

    nc.finalize()
    return nc


def _get_nc():
    if "nc" not in _CACHE:
        _CACHE["nc"] = build_nc()
    return _CACHE["nc"]


def _ensure_axon_profile_hook():
    """Register the NTFF profile hook that this image's antenv lacks."""
    try:
        from antenv.axon_hooks import get_axon_ntff_profile_hook  # noqa: F401
        return
    except ImportError:
        pass
    import types

    import antenv

    mod = types.ModuleType("antenv.axon_hooks")
    _h = {"hook": None}

    def set_axon_ntff_profile_hook(h):
        _h["hook"] = h

    def get_axon_ntff_profile_hook():
        return _h["hook"]

    mod.set_axon_ntff_profile_hook = set_axon_ntff_profile_hook
    mod.get_axon_ntff_profile_hook = get_axon_ntff_profile_hook
    sys.modules["antenv.axon_hooks"] = mod
    antenv.axon_hooks = mod
    try:
        from trn_agent_boot.trn_boot import _ntff_profile_via_ctypes

        set_axon_ntff_profile_hook(
            _ntff_profile_via_ctypes("/opt/axon/libaxon_pjrt.so")
        )
    except Exception as e:  # degrade to no tracing
        print("ntff hook install failed:", e)


def kernel(ref_mels: np.ndarray, mel2ph: np.ndarray, embedding: np.ndarray):
    from concourse.bass_utils import run_bass_kernel_spmd

    nc = _get_nc()

    ref_mels = np.ascontiguousarray(ref_mels, dtype=np.float32)
    mel2ph = np.ascontiguousarray(mel2ph, dtype=np.int32)
    embedding = np.ascontiguousarray(embedding, dtype=np.float32)

    mel_hi = ref_mels.astype(np.float16)
    mel_lo = (ref_mels - mel_hi.astype(np.float32)).astype(np.float16)

    in_maps = []
    for c in range(N_CORES):
        in_maps.append({
            "mel_hi": mel_hi[c * B_LOC:(c + 1) * B_LOC],
            "mel_lo": mel_lo[c * B_LOC:(c + 1) * B_LOC],
            "ids": mel2ph[c * B_LOC:(c + 1) * B_LOC],
            "emb": embedding,
        })

    trace = bool(int(os.environ.get("KERNEL_TRACE", "0")))
    if trace:
        _ensure_axon_profile_hook()
    res = run_bass_kernel_spmd(
        nc, in_maps, core_ids=list(range(N_CORES)), trace=trace,
    )
    _CACHE["last_results"] = res

    z = np.concatenate([res.results[c]["z"] for c in range(N_CORES)], axis=0)
    stats = np.stack([res.results[c]["stats"][0] for c in range(N_CORES)])
    stats = stats.astype(np.float32)

    hist = stats[:, :M].sum(axis=0, dtype=np.float32)
    num = np.float32(stats[:, M].sum(dtype=np.float32))
    den = np.float32(stats[:, M + 1].sum(dtype=np.float32))
    loss = np.float32(np.float32(0.25) * num / den)

    probs = (hist / np.float32(B * S)).astype(np.float32)
    plogp = probs * np.log(probs + np.float32(1e-10), dtype=np.float32)
    perplexity = np.exp(-plogp.sum(dtype=np.float32)).astype(np.float32)

    return z, np.asarray(loss, np.float32), np.asarray(perplexity, np.float32)


if __name__ == "__main__":
    nc = build_nc()
    print("built ok")


# revision 19
# speedup vs baseline: 1.6750x; 1.0103x over previous
"""Trainium2 Bass kernel for LocalStyleAdaptor (segment-pool + VQ codebook).

Reference computation:
  x, cnt = segment-mean-pool(ref_mels [B,T,H], mel2ph [B,T]) -> [B,512,H]
  VQ: indices = argmin_j ||x - e_j||^2 ; quantized = E[indices]
  loss = 0.25 * sum(mean((x-q)^2,-1) * np) / sum(np), np = (sum|x|,-1) > 0
  z = x + sg(q - x)  (numerically == quantized)
  perplexity = exp(-sum(p log(p+1e-10))), p = histogram(indices)/(B*512)

Sharding: data-parallel over batch: 16 batch elems -> 8 cores x 2.
Each core returns z [2,512,256] and stats [1,66] = [hist(64) | loss_num | loss_den].
Host concatenates z, sums stats, computes the two scalars.

Device algorithm (per core, per batch elem):
  Phase 1 (pooling): mel is pre-split on host into fp16 hi + fp16 lo
    (hi+lo reconstructs fp32 to ~22 mantissa bits; fp16 streams through the
    PE at 1 col/cycle vs fp32's effective 1/4). For each 128-frame chunk,
    build one-hot A [128t, 512s] in fp16 on DVE (iota == id-1, per-partition
    scalar compare). Use A slices as matmul *weights*; two moving passes
    [mel_hi | ones] and [mel_lo | zeros] [128t, 257] accumulate into PSUM
    acc_k [128s, 257]: S (sums) + counts (col 256) over 64 chunks.
  Phase 2a (drain): copy S/counts out of PSUM immediately so the next
    batch's pooling can reuse the accumulator banks.
  Phase 2b (VQ): Dn[s,j] = 2*S.e_j - max(cnt,1)*||e_j||^2 via PE
    (argmax_j Dn == argmin_j of reference distances, scale-free).
    First-index argmax via reduce_max + masked-iota reduce_min (ties as jnp).
    mask1 = exact one-hot of argmin; z rows = mask1^T.T @ E via PE.
  Phase 2c: loss columns batched [128,4]; stats accumulated in SBUF,
    one final ones^T @ statacc matmul -> [1,66].
"""

import os
import sys

import numpy as np

for _p in ("/opt/trn_rl_repo", "/root/.axon_site/_ro/trn_rl_repo"):
    if os.path.isdir(_p) and _p not in sys.path:
        sys.path.insert(0, _p)

import concourse.bacc as bacc  # noqa: E402
import concourse.tile as tile  # noqa: E402
from concourse import mybir  # noqa: E402
from concourse.masks import make_identity  # noqa: E402

F32 = mybir.dt.float32
F16 = mybir.dt.float16
I32 = mybir.dt.int32
OP = mybir.AluOpType
AX = mybir.AxisListType
AF = mybir.ActivationFunctionType

B, T, H, M = 16, 8192, 256, 64
S = 512          # MAX_PH segments
N_CORES = 8
B_LOC = B // N_CORES   # 2 batch elems per core
N_CHUNK = T // 128     # 64 chunks of 128 frames
MEGA = 4               # chunks per DMA load
BIG = 1000.0
LEVEL = int(os.environ.get("KERNEL_LEVEL", "99"))

_CACHE = {}


def build_nc():
    nc = bacc.Bacc("TRN2", name="lsa_vq")

    mel_hi = nc.dram_tensor("mel_hi", [B_LOC, T, H], F16, kind="ExternalInput")
    mel_lo = nc.dram_tensor("mel_lo", [B_LOC, T, H], F16, kind="ExternalInput")
    ids = nc.dram_tensor("ids", [B_LOC, T], I32, kind="ExternalInput")
    emb = nc.dram_tensor("emb", [M, H], F32, kind="ExternalInput")
    z_out = nc.dram_tensor("z", [B_LOC, S, H], F32, kind="ExternalOutput")
    st_out = nc.dram_tensor("stats", [1, M + 2], F32, kind="ExternalOutput")

    with tile.TileContext(nc) as tc:
        with (
            tc.tile_pool(name="const", bufs=1) as cpool,
            tc.tile_pool(name="melp", bufs=3) as melp,
            tc.tile_pool(name="ohp", bufs=3) as ohp,
            tc.tile_pool(name="ph2", bufs=2) as ph2,
            tc.tile_pool(name="zsb", bufs=2) as zsbp,
            tc.tile_pool(name="acc", bufs=1, space="PSUM") as accp,
            tc.tile_pool(name="misc", bufs=3, space="PSUM") as miscp,
            tc.tile_pool(name="zps", bufs=1, space="PSUM") as zpsp,
        ):
            # ---------------- ids first (phase-1 critical path) ----------
            ident = cpool.tile([128, 128], F32)
            make_identity(nc, ident[:])

            idsrf_l = []
            for b in range(B_LOC):
                idsr = cpool.tile([64, 128], I32, tag=f"idsr_{b}", name=f"idsr{b}")
                nc.sync.dma_start(
                    out=idsr[:], in_=ids[b].rearrange("(c u) -> c u", c=64)
                )
                idsrf = cpool.tile([64, 128], F32, tag=f"idsrf_{b}", name=f"idsrf{b}")
                nc.vector.tensor_scalar(idsrf[:], idsr[:], -1.0, None, OP.add)
                idsrf_l.append(idsrf)

            ids_t = []
            for b in range(B_LOC):
                tp = miscp.tile([128, 64], F32, tag="tp", name=f"idtp{b}")
                nc.tensor.transpose(
                    out=tp[:], in_=idsrf_l[b][:], identity=ident[:64, :64]
                )
                dst = cpool.tile([128, 64], F32, tag=f"idst_{b}", name=f"idst{b}")
                nc.vector.tensor_copy(dst[:], tp[:])
                ids_t.append(dst)

            iota_s_i = cpool.tile([128, S], I32)
            nc.gpsimd.iota(iota_s_i[:], pattern=[[1, S]], base=0, channel_multiplier=0)
            iota_s = cpool.tile([128, S], F16)
            nc.vector.tensor_copy(iota_s[:], iota_s_i[:])

            iota_j_i = cpool.tile([128, M], I32)
            nc.gpsimd.iota(iota_j_i[:], pattern=[[1, M]], base=0, channel_multiplier=0)
            iota_j = cpool.tile([128, M], F32)
            nc.vector.tensor_copy(iota_j[:], iota_j_i[:])
            iota_jb = cpool.tile([128, M], F32)
            nc.vector.tensor_scalar(iota_jb[:], iota_j_i[:], BIG, None, OP.add)
            identh = cpool.tile([128, 128], F16)
            nc.vector.tensor_copy(identh[:], ident[:])

            ones_col = cpool.tile([128, 1], F32)
            nc.vector.memset(ones_col[:], 1.0)
            ones_row = cpool.tile([1, 128], F32)
            nc.vector.memset(ones_row[:], 1.0)
            statacc = cpool.tile([128, M + 2], F32)
            nc.vector.memset(statacc[:], 0.0)

            # ---------------- embedding tables ----------------
            e_sb = cpool.tile([M, H], F32)
            nc.sync.dma_start(out=e_sb[:], in_=emb[:])
            e_sq = cpool.tile([M, H], F32)
            e2_col = cpool.tile([M, 1], F32)   # ||e_j||^2  [64,1]
            nc.scalar.activation(e_sq[:], e_sb[:], AF.Square, accum_out=e2_col[:])
            e2row_ps = miscp.tile([1, M], F32, tag="tp")
            nc.tensor.transpose(out=e2row_ps[:], in_=e2_col[:], identity=ident[:M, :M])
            e2_row = cpool.tile([1, M], F32)
            nc.vector.tensor_copy(e2_row[:], e2row_ps[:])
            e2bc_ps = miscp.tile([128, M], F32, tag="tp")
            nc.tensor.matmul(out=e2bc_ps[:], lhsT=ones_row[:], rhs=e2_row[:],
                             start=True, stop=True)
            e2_bc = cpool.tile([128, M], F32)
            nc.vector.tensor_copy(e2_bc[:], e2bc_ps[:])

            e_hi16 = cpool.tile([M, H], F16)
            nc.vector.tensor_copy(e_hi16[:], e_sb[:])
            e_lo16 = cpool.tile([M, H], F16)
            nc.vector.tensor_tensor(e_lo16[:], e_sb[:], e_hi16[:], OP.subtract)

            # Ep2 = 2 * E^T halves: [128h, 64j] x2
            ep2 = []
            for h in range(2):
                tp = miscp.tile([128, M], F32, tag="tp", name=f"etp{h}")
                nc.tensor.transpose(
                    out=tp[:], in_=e_sb[:, h * 128:(h + 1) * 128],
                    identity=ident[:M, :M],
                )
                dst = cpool.tile([128, M], F32, tag=f"ep2_{h}", name=f"ep2{h}")
                nc.vector.tensor_scalar(dst[:], tp[:], 2.0, None, OP.mult)
                ep2.append(dst)

            # ---------------- main per-batch work ----------------
            for b in range(B_LOC):
                acc = [
                    accp.tile([128, S], F32, tag=f"acc{k}", name=f"acc{k}_{b}")
                    for k in range(4)
                ]

                # phase 1: pooling
                for mc in range(T // (128 * MEGA)):  # 16 mega chunks
                    rows = slice(mc * 128 * MEGA, (mc + 1) * 128 * MEGA)
                    melth = melp.tile([128, MEGA, H + 1], F16, tag="melth")
                    nc.sync.dma_start(
                        out=melth[:, :, 0:H],
                        in_=mel_hi[b, rows, :].rearrange("(c u) h -> u c h", u=128),
                    )
                    nc.vector.memset(melth[:, :, H:H + 1], 1.0)
                    meltl = melp.tile([128, MEGA, H + 1], F16, tag="meltl")
                    nc.sync.dma_start(
                        out=meltl[:, :, 0:H],
                        in_=mel_lo[b, rows, :].rearrange("(c u) h -> u c h", u=128),
                    )
                    nc.vector.memset(meltl[:, :, H:H + 1], 0.0)
                    for j in range(MEGA):
                        c = mc * MEGA + j
                        a = ohp.tile([128, S], F16, tag="onehot")
                        nc.vector.tensor_scalar(
                            a[:], iota_s[:], ids_t[b][:, c:c + 1], None, OP.is_equal
                        )
                        for k in range(4):
                            nc.tensor.matmul(
                                out=acc[k][:, 0:H + 1],
                                lhsT=a[:, k * 128:(k + 1) * 128],
                                rhs=melth[:, j, :],
                                start=(c == 0), stop=False,
                            )
                            nc.tensor.matmul(
                                out=acc[k][:, 0:H + 1],
                                lhsT=a[:, k * 128:(k + 1) * 128],
                                rhs=meltl[:, j, :],
                                start=False, stop=(c == N_CHUNK - 1),
                            )

                # phase 2a: drain PSUM accumulators ASAP (frees banks for the
                # next batch's pooling matmuls)
                zsb = zsbp.tile([128, 4, H], F32, tag="zsb")
                s_sbs, combs, m1hs = [], [], []
                ccB = ph2.tile([128, 4], F32, tag="ccB")
                ssB = ph2.tile([128, 4], F32, tag="ssB")
                recB = ph2.tile([128, 4], F32, tag="recB")
                mxB = ph2.tile([128, 4], F32, tag="mxB")
                e2selB = ph2.tile([128, 4], F32, tag="e2selB")
                for k in range(4):
                    s_sb = ph2.tile([128, H], F32, tag="s_sb", bufs=8)
                    nc.scalar.copy(s_sb[:], acc[k][:, 0:H])
                    s_sbs.append(s_sb)
                    if LEVEL <= 2:
                        nc.vector.tensor_copy(zsb[:, k, :], s_sb[:])
                        continue
                    s_sq = ph2.tile([128, H], F32, tag="s_sq")
                    nc.scalar.activation(
                        s_sq[:], acc[k][:, 0:H], AF.Square,
                        accum_out=ssB[:, k:k + 1],
                    )
                    nc.vector.tensor_scalar(
                        ccB[:, k:k + 1], acc[k][:, H:H + 1], 1.0, None, OP.max
                    )
                    comb = ph2.tile([128, 2], F32, tag="comb", bufs=8)
                    nc.vector.tensor_scalar(
                        comb[:, 1:2], acc[k][:, H:H + 1], 1.0, None, OP.min
                    )
                    combs.append(comb)

                if LEVEL <= 2:
                    nc.sync.dma_start(
                        out=z_out[b].rearrange("(k u) h -> u k h", k=4), in_=zsb[:]
                    )
                    continue

                nc.vector.reciprocal(recB[:], ccB[:])

                # phase 2b: VQ chain per s-tile
                for k in range(4):
                    s_sb, comb = s_sbs[k], combs[k]
                    st_sb = ph2.tile([128, H], F32, tag="st_sb")
                    for h in range(2):
                        tp = miscp.tile([128, 128], F32, tag="tp", name=f"stp{h}")
                        nc.tensor.transpose(
                            out=tp[:], in_=s_sb[:, h * 128:(h + 1) * 128],
                            identity=ident[:],
                        )
                        nc.scalar.copy(st_sb[:, h * 128:(h + 1) * 128], tp[:])

                    dps = miscp.tile([128, M], F32, tag="tp")
                    nc.tensor.matmul(out=dps[:], lhsT=st_sb[:, 0:128], rhs=ep2[0][:],
                                     start=True, stop=False)
                    nc.tensor.matmul(out=dps[:], lhsT=st_sb[:, 128:256], rhs=ep2[1][:],
                                     start=False, stop=True)
                    cce2 = ph2.tile([128, M], F32, tag="cce2")
                    nc.vector.tensor_scalar(
                        cce2[:], e2_bc[:], ccB[:, k:k + 1], None, OP.mult
                    )
                    dn = ph2.tile([128, M], F32, tag="dn")
                    nc.vector.tensor_tensor(dn[:], dps[:], cce2[:], OP.subtract)

                    # argmax with first-index tie-break:
                    # cand = iota + BIG - BIG*(dn==mx); min(cand) = argmax idx
                    nc.vector.tensor_reduce(mxB[:, k:k + 1], dn[:], AX.X, OP.max)
                    t1 = ph2.tile([128, M], F32, tag="t1")
                    nc.vector.tensor_scalar(
                        t1[:], dn[:], mxB[:, k:k + 1], -BIG, OP.is_equal, OP.mult
                    )
                    cand = ph2.tile([128, M], F32, tag="cand")
                    nc.vector.tensor_tensor(cand[:], t1[:], iota_jb[:], OP.add)
                    idxf = ph2.tile([128, 1], F32, tag="idxf")
                    nc.vector.tensor_reduce(idxf[:], cand[:], AX.X, OP.min)
                    m1h = ph2.tile([128, M], F16, tag="m1h", bufs=8)
                    nc.vector.tensor_scalar(
                        m1h[:], iota_j[:], idxf[:, 0:1], None, OP.is_equal
                    )
                    m1hs.append(m1h)

                    # e2sel = sum(mask1 * E2)
                    esel_s = ph2.tile([128, M], F32, tag="esel_s")
                    nc.vector.tensor_tensor(esel_s[:], m1h[:], e2_bc[:], OP.mult)
                    nc.vector.tensor_reduce(e2selB[:, k:k + 1], esel_s[:], AX.X, OP.add)

                    # z rows = one_hot(idx) @ (E_hi + E_lo)
                    m1t_ps = miscp.tile([M, 128], F16, tag="tp")
                    nc.tensor.transpose(out=m1t_ps[:], in_=m1h[:],
                                        identity=identh[:])
                    m1t_sb = ph2.tile([M, 128], F16, tag="m1t_sb")
                    nc.scalar.copy(m1t_sb[:], m1t_ps[:])
                    zps = zpsp.tile([128, H], F32, tag="zps")
                    nc.tensor.matmul(out=zps[:], lhsT=m1t_sb[:], rhs=e_hi16[:],
                                     start=True, stop=False)
                    nc.tensor.matmul(out=zps[:], lhsT=m1t_sb[:], rhs=e_lo16[:],
                                     start=False, stop=True)
                    nc.scalar.copy(zsb[:, k, :], zps[:])

                nc.sync.dma_start(
                    out=z_out[b].rearrange("(k u) h -> u k h", k=4), in_=zsb[:]
                )

                # phase 2c: batched loss columns
                # e_lat = (SS*rec^2 - (mx + cc*e2sel)*rec + e2sel)/H
                u1 = ph2.tile([128, 4], F32, tag="u1")
                nc.vector.tensor_tensor(u1[:], ccB[:], e2selB[:], OP.mult)
                u2 = ph2.tile([128, 4], F32, tag="u2")
                nc.vector.tensor_tensor(u2[:], u1[:], mxB[:], OP.add)
                b3 = ph2.tile([128, 4], F32, tag="b3")
                nc.vector.tensor_tensor(b3[:], u2[:], recB[:], OP.mult)
                a1 = ph2.tile([128, 4], F32, tag="a1")
                nc.vector.tensor_tensor(a1[:], ssB[:], recB[:], OP.mult)
                a2 = ph2.tile([128, 4], F32, tag="a2")
                nc.vector.tensor_tensor(a2[:], a1[:], recB[:], OP.mult)
                c1 = ph2.tile([128, 4], F32, tag="c1")
                nc.vector.tensor_tensor(c1[:], a2[:], b3[:], OP.subtract)
                c2 = ph2.tile([128, 4], F32, tag="c2")
                nc.vector.tensor_tensor(c2[:], c1[:], e2selB[:], OP.add)
                c3 = ph2.tile([128, 4], F32, tag="c3")
                nc.vector.tensor_scalar(c3[:], c2[:], 1.0 / H, None, OP.mult)
                for k in range(4):
                    nc.vector.tensor_tensor(
                        combs[k][:, 0:1], c3[:, k:k + 1],
                        combs[k][:, 1:2], OP.mult,
                    )
                    nc.vector.tensor_tensor(
                        statacc[:, 0:M], statacc[:, 0:M], m1hs[k][:], OP.add
                    )
                    nc.vector.tensor_tensor(
                        statacc[:, M:M + 2], statacc[:, M:M + 2], combs[k][:], OP.add
                    )

            if LEVEL >= 3:
                stats_ps = miscp.tile([1, M + 2], F32, tag="tp")
                nc.tensor.matmul(out=stats_ps[:], lhsT=ones_col[:], rhs=statacc[:],
                                 start=True, stop=True)
                stats_sb = cpool.tile([1, M + 2], F32)
                nc.vector.tensor_copy(stats_sb[:], stats_ps[:])
                nc.sync.dma_start(out=st_out[:], in_=stats_sb[:])

    nc.finalize()
    return nc


def _get_nc():
    if "nc" not in _CACHE:
        _CACHE["nc"] = build_nc()
    return _CACHE["nc"]


def _ensure_axon_profile_hook():
    """Register the NTFF profile hook that this image's antenv lacks."""
    try:
        from antenv.axon_hooks import get_axon_ntff_profile_hook  # noqa: F401
        return
    except ImportError:
        pass
    import types

    import antenv

    mod = types.ModuleType("antenv.axon_hooks")
    _h = {"hook": None}

    def set_axon_ntff_profile_hook(h):
        _h["hook"] = h

    def get_axon_ntff_profile_hook():
        return _h["hook"]

    mod.set_axon_ntff_profile_hook = set_axon_ntff_profile_hook
    mod.get_axon_ntff_profile_hook = get_axon_ntff_profile_hook
    sys.modules["antenv.axon_hooks"] = mod
    antenv.axon_hooks = mod
    try:
        from trn_agent_boot.trn_boot import _ntff_profile_via_ctypes

        set_axon_ntff_profile_hook(
            _ntff_profile_via_ctypes("/opt/axon/libaxon_pjrt.so")
        )
    except Exception as e:  # degrade to no tracing
        print("ntff hook install failed:", e)


def kernel(ref_mels: np.ndarray, mel2ph: np.ndarray, embedding: np.ndarray):
    from concourse.bass_utils import run_bass_kernel_spmd

    nc = _get_nc()

    ref_mels = np.ascontiguousarray(ref_mels, dtype=np.float32)
    mel2ph = np.ascontiguousarray(mel2ph, dtype=np.int32)
    embedding = np.ascontiguousarray(embedding, dtype=np.float32)

    mel_hi = ref_mels.astype(np.float16)
    mel_lo = (ref_mels - mel_hi.astype(np.float32)).astype(np.float16)

    in_maps = []
    for c in range(N_CORES):
        in_maps.append({
            "mel_hi": mel_hi[c * B_LOC:(c + 1) * B_LOC],
            "mel_lo": mel_lo[c * B_LOC:(c + 1) * B_LOC],
            "ids": mel2ph[c * B_LOC:(c + 1) * B_LOC],
            "emb": embedding,
        })

    trace = bool(int(os.environ.get("KERNEL_TRACE", "0")))
    if trace:
        _ensure_axon_profile_hook()
    res = run_bass_kernel_spmd(
        nc, in_maps, core_ids=list(range(N_CORES)), trace=trace,
    )
    _CACHE["last_results"] = res

    z = np.concatenate([res.results[c]["z"] for c in range(N_CORES)], axis=0)
    stats = np.stack([res.results[c]["stats"][0] for c in range(N_CORES)])
    stats = stats.astype(np.float32)

    hist = stats[:, :M].sum(axis=0, dtype=np.float32)
    num = np.float32(stats[:, M].sum(dtype=np.float32))
    den = np.float32(stats[:, M + 1].sum(dtype=np.float32))
    loss = np.float32(np.float32(0.25) * num / den)

    probs = (hist / np.float32(B * S)).astype(np.float32)
    plogp = probs * np.log(probs + np.float32(1e-10), dtype=np.float32)
    perplexity = np.exp(-plogp.sum(dtype=np.float32)).astype(np.float32)

    return z, np.asarray(loss, np.float32), np.asarray(perplexity, np.float32)


if __name__ == "__main__":
    nc = build_nc()
    print("built ok")


# revision 21
# speedup vs baseline: 1.7218x; 1.0280x over previous
"""Trainium2 Bass kernel for LocalStyleAdaptor (segment-pool + VQ codebook).

Reference computation:
  x, cnt = segment-mean-pool(ref_mels [B,T,H], mel2ph [B,T]) -> [B,512,H]
  VQ: indices = argmin_j ||x - e_j||^2 ; quantized = E[indices]
  loss = 0.25 * sum(mean((x-q)^2,-1) * np) / sum(np), np = (sum|x|,-1) > 0
  z = x + sg(q - x)  (numerically == quantized)
  perplexity = exp(-sum(p log(p+1e-10))), p = histogram(indices)/(B*512)

Sharding: data-parallel over batch: 16 batch elems -> 8 cores x 2.
Each core returns z [2,512,256] and stats [1,66] = [hist(64) | loss_num | loss_den].
Host concatenates z, sums stats, computes the two scalars.

Device algorithm (per core, per batch elem):
  Phase 1 (pooling): mel is pre-split on host into fp16 hi + fp16 lo
    (hi+lo reconstructs fp32 to ~22 mantissa bits; fp16 streams through the
    PE at 1 col/cycle vs fp32's effective 1/4). For each 128-frame chunk,
    build one-hot A [128t, 512s] in fp16 on DVE (iota == id-1, per-partition
    scalar compare). Use A slices as matmul *weights*; two moving passes
    [mel_hi | ones] and [mel_lo | zeros] [128t, 257] accumulate into PSUM
    acc_k [128s, 257]: S (sums) + counts (col 256) over 64 chunks.
  Phase 2a (drain): copy S/counts out of PSUM immediately so the next
    batch's pooling can reuse the accumulator banks.
  Phase 2b (VQ): Dn[s,j] = 2*S.e_j - max(cnt,1)*||e_j||^2 via PE
    (argmax_j Dn == argmin_j of reference distances, scale-free).
    First-index argmax via reduce_max + masked-iota reduce_min (ties as jnp).
    mask1 = exact one-hot of argmin; z rows = mask1^T.T @ E via PE.
  Phase 2c: loss columns batched [128,4]; stats accumulated in SBUF,
    one final ones^T @ statacc matmul -> [1,66].
"""

import os
import sys

import numpy as np

for _p in ("/opt/trn_rl_repo", "/root/.axon_site/_ro/trn_rl_repo"):
    if os.path.isdir(_p) and _p not in sys.path:
        sys.path.insert(0, _p)

import concourse.bacc as bacc  # noqa: E402
import concourse.tile as tile  # noqa: E402
from concourse import mybir  # noqa: E402
from concourse.masks import make_identity  # noqa: E402

F32 = mybir.dt.float32
F16 = mybir.dt.float16
I32 = mybir.dt.int32
OP = mybir.AluOpType
AX = mybir.AxisListType
AF = mybir.ActivationFunctionType

B, T, H, M = 16, 8192, 256, 64
S = 512          # MAX_PH segments
N_CORES = 8
B_LOC = B // N_CORES   # 2 batch elems per core
N_CHUNK = T // 128     # 64 chunks of 128 frames
MEGA = 4               # chunks per DMA load
BIG = 1000.0
LEVEL = int(os.environ.get("KERNEL_LEVEL", "99"))

_CACHE = {}


def build_nc():
    nc = bacc.Bacc("TRN2", name="lsa_vq")

    mel_hi = nc.dram_tensor("mel_hi", [B_LOC, T, H], F16, kind="ExternalInput")
    mel_lo = nc.dram_tensor("mel_lo", [B_LOC, T, H], F16, kind="ExternalInput")
    ids = nc.dram_tensor("ids", [B_LOC, T], I32, kind="ExternalInput")
    emb = nc.dram_tensor("emb", [M, H], F32, kind="ExternalInput")
    z_out = nc.dram_tensor("z", [B_LOC, S, H], F32, kind="ExternalOutput")
    st_out = nc.dram_tensor("stats", [1, M + 2], F32, kind="ExternalOutput")

    with tile.TileContext(nc) as tc:
        with (
            tc.tile_pool(name="const", bufs=1) as cpool,
            tc.tile_pool(name="melp", bufs=3) as melp,
            tc.tile_pool(name="ohp", bufs=3) as ohp,
            tc.tile_pool(name="ph2", bufs=2) as ph2,
            tc.tile_pool(name="zsb", bufs=2) as zsbp,
            tc.tile_pool(name="acc", bufs=1, space="PSUM") as accp,
            tc.tile_pool(name="misc", bufs=2, space="PSUM") as miscp,
            tc.tile_pool(name="zps", bufs=1, space="PSUM") as zpsp,
        ):
            # ---------------- ids first (phase-1 critical path) ----------
            ident = cpool.tile([128, 128], F32)
            make_identity(nc, ident[:])

            idsrf_l = []
            for b in range(B_LOC):
                idsr = cpool.tile([64, 128], I32, tag=f"idsr_{b}", name=f"idsr{b}")
                nc.sync.dma_start(
                    out=idsr[:], in_=ids[b].rearrange("(c u) -> c u", c=64)
                )
                idsrf = cpool.tile([64, 128], F32, tag=f"idsrf_{b}", name=f"idsrf{b}")
                nc.vector.tensor_scalar(idsrf[:], idsr[:], -1.0, None, OP.add)
                idsrf_l.append(idsrf)

            ids_t = []
            for b in range(B_LOC):
                tp = miscp.tile([128, 64], F32, tag="tp", name=f"idtp{b}")
                nc.tensor.transpose(
                    out=tp[:], in_=idsrf_l[b][:], identity=ident[:64, :64]
                )
                dst = cpool.tile([128, 64], F32, tag=f"idst_{b}", name=f"idst{b}")
                nc.vector.tensor_copy(dst[:], tp[:])
                ids_t.append(dst)

            iota_s_i = cpool.tile([128, S], I32)
            nc.gpsimd.iota(iota_s_i[:], pattern=[[1, S]], base=0, channel_multiplier=0)
            iota_s = cpool.tile([128, S], F16)
            nc.vector.tensor_copy(iota_s[:], iota_s_i[:])

            iota_j_i = cpool.tile([128, M], I32)
            nc.gpsimd.iota(iota_j_i[:], pattern=[[1, M]], base=0, channel_multiplier=0)
            iota_j = cpool.tile([128, M], F32)
            nc.vector.tensor_copy(iota_j[:], iota_j_i[:])
            iota_jb = cpool.tile([128, M], F32)
            nc.vector.tensor_scalar(iota_jb[:], iota_j_i[:], BIG, None, OP.add)
            identh = cpool.tile([128, 128], F16)
            nc.vector.tensor_copy(identh[:], ident[:])

            ones_col = cpool.tile([128, 1], F32)
            nc.vector.memset(ones_col[:], 1.0)
            ones_row = cpool.tile([1, 128], F32)
            nc.vector.memset(ones_row[:], 1.0)
            statacc = cpool.tile([128, M + 2], F32)
            nc.vector.memset(statacc[:], 0.0)

            # ---------------- embedding tables ----------------
            e_sb = cpool.tile([M, H], F32)
            nc.sync.dma_start(out=e_sb[:], in_=emb[:])
            e_sq = cpool.tile([M, H], F32)
            e2_col = cpool.tile([M, 1], F32)   # ||e_j||^2  [64,1]
            nc.scalar.activation(e_sq[:], e_sb[:], AF.Square, accum_out=e2_col[:])
            e2row_ps = miscp.tile([1, M], F32, tag="tp")
            nc.tensor.transpose(out=e2row_ps[:], in_=e2_col[:], identity=ident[:M, :M])
            e2_row = cpool.tile([1, M], F32)
            nc.vector.tensor_copy(e2_row[:], e2row_ps[:])
            e2bc_ps = miscp.tile([128, M], F32, tag="tp")
            nc.tensor.matmul(out=e2bc_ps[:], lhsT=ones_row[:], rhs=e2_row[:],
                             start=True, stop=True)
            e2_bc = cpool.tile([128, M], F32)
            nc.vector.tensor_copy(e2_bc[:], e2bc_ps[:])

            e_hi16 = cpool.tile([M, H], F16)
            nc.vector.tensor_copy(e_hi16[:], e_sb[:])
            e_lo16 = cpool.tile([M, H], F16)
            nc.vector.tensor_tensor(e_lo16[:], e_sb[:], e_hi16[:], OP.subtract)

            # Ep2 = 2 * E^T halves: [128h, 64j] x2
            ep2 = []
            for h in range(2):
                tp = miscp.tile([128, M], F32, tag="tp", name=f"etp{h}")
                nc.tensor.transpose(
                    out=tp[:], in_=e_sb[:, h * 128:(h + 1) * 128],
                    identity=ident[:M, :M],
                )
                dst = cpool.tile([128, M], F32, tag=f"ep2_{h}", name=f"ep2{h}")
                nc.vector.tensor_scalar(dst[:], tp[:], 2.0, None, OP.mult)
                ep2.append(dst)

            # ---------------- main per-batch work ----------------
            for b in range(B_LOC):
                acc = [
                    accp.tile([128, S], F32, tag=f"acc{k}", name=f"acc{k}_{b}")
                    for k in range(4)
                ]

                # phase 1: pooling
                for mc in range(T // (128 * MEGA)):  # 16 mega chunks
                    rows = slice(mc * 128 * MEGA, (mc + 1) * 128 * MEGA)
                    melth = melp.tile([128, MEGA, H + 1], F16, tag="melth")
                    nc.sync.dma_start(
                        out=melth[:, :, 0:H],
                        in_=mel_hi[b, rows, :].rearrange("(c u) h -> u c h", u=128),
                    )
                    nc.vector.memset(melth[:, :, H:H + 1], 1.0)
                    meltl = melp.tile([128, MEGA, H + 1], F16, tag="meltl")
                    nc.sync.dma_start(
                        out=meltl[:, :, 0:H],
                        in_=mel_lo[b, rows, :].rearrange("(c u) h -> u c h", u=128),
                    )
                    nc.vector.memset(meltl[:, :, H:H + 1], 0.0)
                    for j in range(MEGA):
                        c = mc * MEGA + j
                        a = ohp.tile([128, S], F16, tag="onehot")
                        nc.vector.tensor_scalar(
                            a[:], iota_s[:], ids_t[b][:, c:c + 1], None, OP.is_equal
                        )
                        for k in range(4):
                            nc.tensor.matmul(
                                out=acc[k][:, 0:H + 1],
                                lhsT=a[:, k * 128:(k + 1) * 128],
                                rhs=melth[:, j, :],
                                start=(c == 0), stop=False,
                            )
                            nc.tensor.matmul(
                                out=acc[k][:, 0:H + 1],
                                lhsT=a[:, k * 128:(k + 1) * 128],
                                rhs=meltl[:, j, :],
                                start=False, stop=(c == N_CHUNK - 1),
                            )

                # phase 2a: drain PSUM accumulators ASAP (frees banks for the
                # next batch's pooling matmuls)
                zsb = zsbp.tile([128, 4, H], F32, tag="zsb")
                s_sbs, combs, m1hs = [], [], []
                ccB = ph2.tile([128, 4], F32, tag="ccB")
                ssB = ph2.tile([128, 4], F32, tag="ssB")
                recB = ph2.tile([128, 4], F32, tag="recB")
                mxB = ph2.tile([128, 4], F32, tag="mxB")
                e2selB = ph2.tile([128, 4], F32, tag="e2selB")
                for k in range(4):
                    s_sb = ph2.tile([128, H], F32, tag="s_sb", bufs=8)
                    nc.scalar.copy(s_sb[:], acc[k][:, 0:H])
                    s_sbs.append(s_sb)
                    if LEVEL <= 2:
                        nc.vector.tensor_copy(zsb[:, k, :], s_sb[:])
                        continue
                    s_sq = ph2.tile([128, H], F32, tag="s_sq")
                    nc.scalar.activation(
                        s_sq[:], acc[k][:, 0:H], AF.Square,
                        accum_out=ssB[:, k:k + 1],
                    )
                    nc.vector.tensor_scalar(
                        ccB[:, k:k + 1], acc[k][:, H:H + 1], 1.0, None, OP.max
                    )
                    comb = ph2.tile([128, 2], F32, tag="comb", bufs=8)
                    nc.vector.tensor_scalar(
                        comb[:, 1:2], acc[k][:, H:H + 1], 1.0, None, OP.min
                    )
                    combs.append(comb)

                if LEVEL <= 2:
                    nc.sync.dma_start(
                        out=z_out[b].rearrange("(k u) h -> u k h", k=4), in_=zsb[:]
                    )
                    continue

                nc.vector.reciprocal(recB[:], ccB[:])

                # phase 2b-1 (PE dense): S^T transposes + D matmuls
                dps_all = miscp.tile([128, 4, M], F32, tag="dpsall", bufs=1,
                                     name=f"dpsall{b}")
                for k in range(4):
                    s_sb = s_sbs[k]
                    st_sb = ph2.tile([128, H], F32, tag="st_sb", bufs=4,
                                     name=f"st{k}")
                    for h in range(2):
                        tp = miscp.tile([128, 128], F32, tag="tp", name=f"stp{h}")
                        nc.tensor.transpose(
                            out=tp[:], in_=s_sb[:, h * 128:(h + 1) * 128],
                            identity=ident[:],
                        )
                        nc.scalar.copy(st_sb[:, h * 128:(h + 1) * 128], tp[:])
                    nc.tensor.matmul(out=dps_all[:, k, :], lhsT=st_sb[:, 0:128],
                                     rhs=ep2[0][:], start=True, stop=False)
                    nc.tensor.matmul(out=dps_all[:, k, :], lhsT=st_sb[:, 128:256],
                                     rhs=ep2[1][:], start=False, stop=True)

                # phase 2b-2 (DVE): argmax chains.
                # cce2/dn batched [128,4,M]: the single dn subtract reads the
                # whole dps_all tile -> depends on all 4 D matmuls (avoids a
                # PSUM same-bank PE-write/DVE-read race on slices).
                cce2f = ph2.tile([128, 4, M], F32, tag="cce2f")
                for k in range(4):
                    nc.vector.tensor_scalar(
                        cce2f[:, k, :], e2_bc[:], ccB[:, k:k + 1], None, OP.mult
                    )
                dnf = ph2.tile([128, 4, M], F32, tag="dnf")
                nc.vector.tensor_tensor(dnf[:], dps_all[:], cce2f[:], OP.subtract)
                nc.vector.tensor_reduce(mxB[:], dnf[:], AX.X, OP.max)
                for k in range(4):
                    # cand = iota + BIG - BIG*(dn==mx); min(cand) = argmax idx
                    t1 = ph2.tile([128, M], F32, tag="t1", bufs=4, name=f"t1_{k}")
                    nc.vector.tensor_scalar(
                        t1[:], dnf[:, k, :], mxB[:, k:k + 1], -BIG,
                        OP.is_equal, OP.mult
                    )
                    cand = ph2.tile([128, M], F32, tag="cand", bufs=4,
                                    name=f"cand{k}")
                    nc.vector.tensor_tensor(cand[:], t1[:], iota_jb[:], OP.add)
                    idxf = ph2.tile([128, 1], F32, tag="idxf", bufs=4,
                                    name=f"idxf{k}")
                    nc.vector.tensor_reduce(idxf[:], cand[:], AX.X, OP.min)
                    m1h = ph2.tile([128, M], F16, tag="m1h", bufs=8,
                                   name=f"m1h{k}")
                    nc.vector.tensor_scalar(
                        m1h[:], iota_j[:], idxf[:, 0:1], None, OP.is_equal
                    )
                    m1hs.append(m1h)

                # phase 2b-3 (PE): z gather, DMA per s-tile
                for k in range(4):
                    m1t_ps = miscp.tile([M, 128], F16, tag="tp", name=f"m1t{k}")
                    nc.tensor.transpose(out=m1t_ps[:], in_=m1hs[k][:],
                                        identity=identh[:])
                    m1t_sb = ph2.tile([M, 128], F16, tag="m1t_sb", name=f"m1ts{k}")
                    nc.scalar.copy(m1t_sb[:], m1t_ps[:])
                    zps = zpsp.tile([128, H], F32, tag="zps", name=f"zps{k}")
                    nc.tensor.matmul(out=zps[:], lhsT=m1t_sb[:], rhs=e_hi16[:],
                                     start=True, stop=False)
                    nc.tensor.matmul(out=zps[:], lhsT=m1t_sb[:], rhs=e_lo16[:],
                                     start=False, stop=True)
                    nc.scalar.copy(zsb[:, k, :], zps[:])
                    nc.sync.dma_start(
                        out=z_out[b, k * 128:(k + 1) * 128, :], in_=zsb[:, k, :]
                    )

                # phase 2b-4 (DVE): e2sel = sum(mask1 * E2)
                for k in range(4):
                    esel_s = ph2.tile([128, M], F32, tag="esel_s", name=f"es{k}")
                    nc.vector.tensor_tensor(esel_s[:], m1hs[k][:], e2_bc[:], OP.mult)
                    nc.vector.tensor_reduce(e2selB[:, k:k + 1], esel_s[:], AX.X,
                                            OP.add)

                # phase 2c: batched loss columns
                # e_lat = (SS*rec^2 - (mx + cc*e2sel)*rec + e2sel)/H
                u1 = ph2.tile([128, 4], F32, tag="u1")
                nc.vector.tensor_tensor(u1[:], ccB[:], e2selB[:], OP.mult)
                u2 = ph2.tile([128, 4], F32, tag="u2")
                nc.vector.tensor_tensor(u2[:], u1[:], mxB[:], OP.add)
                b3 = ph2.tile([128, 4], F32, tag="b3")
                nc.vector.tensor_tensor(b3[:], u2[:], recB[:], OP.mult)
                a1 = ph2.tile([128, 4], F32, tag="a1")
                nc.vector.tensor_tensor(a1[:], ssB[:], recB[:], OP.mult)
                a2 = ph2.tile([128, 4], F32, tag="a2")
                nc.vector.tensor_tensor(a2[:], a1[:], recB[:], OP.mult)
                c1 = ph2.tile([128, 4], F32, tag="c1")
                nc.vector.tensor_tensor(c1[:], a2[:], b3[:], OP.subtract)
                c2 = ph2.tile([128, 4], F32, tag="c2")
                nc.vector.tensor_tensor(c2[:], c1[:], e2selB[:], OP.add)
                c3 = ph2.tile([128, 4], F32, tag="c3")
                nc.vector.tensor_scalar(c3[:], c2[:], 1.0 / H, None, OP.mult)
                for k in range(4):
                    nc.vector.tensor_tensor(
                        combs[k][:, 0:1], c3[:, k:k + 1],
                        combs[k][:, 1:2], OP.mult,
                    )
                    nc.vector.tensor_tensor(
                        statacc[:, 0:M], statacc[:, 0:M], m1hs[k][:], OP.add
                    )
                    nc.vector.tensor_tensor(
                        statacc[:, M:M + 2], statacc[:, M:M + 2], combs[k][:], OP.add
                    )

            if LEVEL >= 3:
                stats_ps = miscp.tile([1, M + 2], F32, tag="tp")
                nc.tensor.matmul(out=stats_ps[:], lhsT=ones_col[:], rhs=statacc[:],
                                 start=True, stop=True)
                stats_sb = cpool.tile([1, M + 2], F32)
                nc.vector.tensor_copy(stats_sb[:], stats_ps[:])
                nc.sync.dma_start(out=st_out[:], in_=stats_sb[:])

    nc.finalize()
    return nc


def _get_nc():
    if "nc" not in _CACHE:
        _CACHE["nc"] = build_nc()
    return _CACHE["nc"]


def _ensure_axon_profile_hook():
    """Register the NTFF profile hook that this image's antenv lacks."""
    try:
        from antenv.axon_hooks import get_axon_ntff_profile_hook  # noqa: F401
        return
    except ImportError:
        pass
    import types

    import antenv

    mod = types.ModuleType("antenv.axon_hooks")
    _h = {"hook": None}

    def set_axon_ntff_profile_hook(h):
        _h["hook"] = h

    def get_axon_ntff_profile_hook():
        return _h["hook"]

    mod.set_axon_ntff_profile_hook = set_axon_ntff_profile_hook
    mod.get_axon_ntff_profile_hook = get_axon_ntff_profile_hook
    sys.modules["antenv.axon_hooks"] = mod
    antenv.axon_hooks = mod
    try:
        from trn_agent_boot.trn_boot import _ntff_profile_via_ctypes

        set_axon_ntff_profile_hook(
            _ntff_profile_via_ctypes("/opt/axon/libaxon_pjrt.so")
        )
    except Exception as e:  # degrade to no tracing
        print("ntff hook install failed:", e)


def kernel(ref_mels: np.ndarray, mel2ph: np.ndarray, embedding: np.ndarray):
    from concourse.bass_utils import run_bass_kernel_spmd

    nc = _get_nc()

    ref_mels = np.ascontiguousarray(ref_mels, dtype=np.float32)
    mel2ph = np.ascontiguousarray(mel2ph, dtype=np.int32)
    embedding = np.ascontiguousarray(embedding, dtype=np.float32)

    mel_hi = ref_mels.astype(np.float16)
    mel_lo = (ref_mels - mel_hi.astype(np.float32)).astype(np.float16)

    in_maps = []
    for c in range(N_CORES):
        in_maps.append({
            "mel_hi": mel_hi[c * B_LOC:(c + 1) * B_LOC],
            "mel_lo": mel_lo[c * B_LOC:(c + 1) * B_LOC],
            "ids": mel2ph[c * B_LOC:(c + 1) * B_LOC],
            "emb": embedding,
        })

    trace = bool(int(os.environ.get("KERNEL_TRACE", "0")))
    if trace:
        _ensure_axon_profile_hook()
    res = run_bass_kernel_spmd(
        nc, in_maps, core_ids=list(range(N_CORES)), trace=trace,
    )
    _CACHE["last_results"] = res

    z = np.concatenate([res.results[c]["z"] for c in range(N_CORES)], axis=0)
    stats = np.stack([res.results[c]["stats"][0] for c in range(N_CORES)])
    stats = stats.astype(np.float32)

    hist = stats[:, :M].sum(axis=0, dtype=np.float32)
    num = np.float32(stats[:, M].sum(dtype=np.float32))
    den = np.float32(stats[:, M + 1].sum(dtype=np.float32))
    loss = np.float32(np.float32(0.25) * num / den)

    probs = (hist / np.float32(B * S)).astype(np.float32)
    plogp = probs * np.log(probs + np.float32(1e-10), dtype=np.float32)
    perplexity = np.exp(-plogp.sum(dtype=np.float32)).astype(np.float32)

    return z, np.asarray(loss, np.float32), np.asarray(perplexity, np.float32)


if __name__ == "__main__":
    nc = build_nc()
    print("built ok")
